# revision 1
# baseline (speedup 1.0000x reference)
"""Trainium2 Bass kernel for nn_C2SModel (code2seq-style model), v2.

Optimizations over baseline:
  - t=0 LSTM step collapsed to a 512-entry vocab table (h1,c1), built on
    device, DMA-gathered per span.
  - Cell elementwise via scalar_tensor_tensor (4x DVE mode).
  - Phase C (fc + attention) pipelined into the span loop; exp computed
    via tanh identity to keep one activation table set.
  - Dynamic NP (pad to actual max contexts per core).
  - Batched gathers (1024-row calls).
"""
import os
import numpy as np
import ml_dtypes
from contextlib import ExitStack

BF16 = ml_dtypes.bfloat16
T0_TABLE = os.environ.get("NO_T0_TABLE", "") != "1"

# ---- problem constants (hardcoded per contract) ---------------------------
N_CTX = 65536
B = 512
E = 128
H = 128
T = 9
SUB_V = 50000
NODE_V = 512
DEC = 320
OUT_D = 10000
N_CORES = 8
SPC = B // N_CORES            # 64 samples per core
SPLIT = 32768                 # subtoken table split (int16 gather indices)
GROUP = 4                     # ctx-tiles of 128 per gather call group
OUT_NCH = 500                 # final matmul N-chunk (20 chunks of 500)


# ---- host-side prep -------------------------------------------------------

def _wrap_idx(ids):
    ids = np.asarray(ids, np.int16)
    assert len(ids) % 16 == 0
    w16 = ids.reshape(-1, 16).T
    return np.tile(w16, (8, 1)).copy()


def _pad_to(x, n, val):
    out = np.full((n,) + x.shape[1:], val, x.dtype)
    out[: len(x)] = x
    return out


def _core_raw(k, inp, NP):
    """Per-core raw occurrence lists and context data."""
    NT128 = NP // 128
    indices = inp["indices"]
    s = int(np.searchsorted(indices, k * SPC, "left"))
    e = int(np.searchsorted(indices, (k + 1) * SPC, "left"))
    nk = e - s
    assert nk <= NP, f"core {k}: {nk} contexts exceed NP={NP}"
    samp = _pad_to(indices[s:e].astype(np.int32) - k * SPC, NP, -1)
    pth = _pad_to(inp["paths"][s:e].astype(np.int16), NP, 0)
    d = {"samp": samp, "paths": pth, "nk": nk}
    for side in ("ll", "rl"):
        idxs_g = inp[f"{side}_indices"]
        subs_g = inp[f"{side}_subtokens"]
        o_s = int(np.searchsorted(idxs_g, s, "left"))
        o_e = int(np.searchsorted(idxs_g, e, "left"))
        subs = subs_g[o_s:o_e].astype(np.int32)
        ctxs = idxs_g[o_s:o_e].astype(np.int32) - s
        tb = np.searchsorted(ctxs, np.arange(0, NP + 128, 128))
        per_tile = {"lo": [], "hi": []}
        for t in range(NT128):
            sl = slice(tb[t], tb[t + 1])
            tsub, tctx = subs[sl], ctxs[sl] - t * 128
            m = tsub < SPLIT
            per_tile["lo"].append((tsub[m], tctx[m]))
            per_tile["hi"].append((tsub[~m] - SPLIT, tctx[~m]))
        d[side] = per_tile
    return d


def prep_all(inp):
    """Returns (meta, per_core_data)."""
    indices = inp["indices"]
    counts = [
        int(np.searchsorted(indices, (k + 1) * SPC, "left"))
        - int(np.searchsorted(indices, k * SPC, "left"))
        for k in range(N_CORES)
    ]
    NP = ((max(counts) + 127) // 128) * 128   # pad to 128 multiple
    NT128 = NP // 128
    raws = [_core_raw(k, inp, NP) for k in range(N_CORES)]
    nb = {}
    for side in ("ll", "rl"):
        for cls in ("lo", "hi"):
            nb[(side, cls)] = np.array(
                [
                    max((len(r[side][cls][t][0]) + 127) // 128 for r in raws)
                    for t in range(NT128)
                ],
                np.int32,
            )
    meta = {"nb": nb, "np": NP}
    SPANS = [(n, min(1024, NP - n)) for n in range(0, NP, 1024)]

    cores = []
    for r in raws:
        d = {"samp": r["samp"], "nk": r["nk"]}
        pth = r["paths"]
        xidx = np.concatenate(
            [pth[n0 : n0 + W].T.reshape(-1) for (n0, W) in SPANS]
        )
        d["xidx"] = _wrap_idx(xidx)
        d["v0idx"] = _wrap_idx(pth[:, 0].copy())
        d["v8idx"] = _wrap_idx(pth[:, T - 1].copy())
        d["sampid"] = np.ascontiguousarray(
            r["samp"].reshape(NT128, 128).T.astype(np.float32)
        )
        for side in ("ll", "rl"):
            for cls in ("lo", "hi"):
                subs_s, ctxs_s = [], []
                for t in range(NT128):
                    ts_, tc_ = r[side][cls][t]
                    n = nb[(side, cls)][t] * 128
                    subs_s.append(_pad_to(ts_.astype(np.int32), n, 0))
                    ctxs_s.append(_pad_to(tc_.astype(np.int32), n, -1))
                subs_s = np.concatenate(subs_s) if subs_s else np.zeros(0, np.int32)
                ctxs_s = np.concatenate(ctxs_s) if ctxs_s else np.zeros(0, np.int32)
                d[f"{side}_{cls}_idx"] = _wrap_idx(subs_s.astype(np.int16))
                d[f"{side}_{cls}_ctx"] = np.ascontiguousarray(
                    ctxs_s.reshape(-1, 128).T.astype(np.float32)
                )
        cores.append(d)
    return meta, cores


def prep_shared(inp):
    """Replicated (same for all cores) tensors."""
    sub = np.asarray(inp["subtoken_emb"], np.float32)
    node = np.asarray(inp["node_emb"], np.float32)
    sh = {
        "sub_lo": sub[:SPLIT].astype(BF16),
        "sub_hi": sub[SPLIT:].astype(BF16),
        "node_t": node.astype(BF16),
        "nodeT": np.ascontiguousarray(node.T).astype(BF16),  # [128 e, 512 v]
        "ramp": np.tile(np.arange(128, dtype=np.float32), (128, 1)).astype(BF16),
        "iotap": np.tile(
            np.arange(128, dtype=np.float32)[:, None], (1, 8)
        ),  # [128, 8] f32, col j = partition idx
        "a_bc": np.tile(np.asarray(inp["a"], np.float32), (128, 1)).astype(BF16),
    }
    def _sig(x):
        return 1.0 / (1.0 + np.exp(-x))

    for d in ("f", "b"):
        # t=0 LSTM step for all 512 path-node vocab entries (h=c=0):
        # c1 = sig(i)*tanh(g); h1 = sig(o)*tanh(c1)   (gate order i,f,g,o)
        wih = np.asarray(inp[f"w_ih_{d}"], np.float32)
        bias0 = (np.asarray(inp[f"b_ih_{d}"], np.float32)
                 + np.asarray(inp[f"b_hh_{d}"], np.float32))
        g0 = node @ wih.T + bias0                        # [512, 512]
        c1 = _sig(g0[:, 0:128]) * np.tanh(g0[:, 256:384])
        h1 = _sig(g0[:, 384:512]) * np.tanh(c1)
        sh[f"tbl_{d}h"] = np.ascontiguousarray(h1).astype(BF16)
        sh[f"tbl_{d}c"] = np.ascontiguousarray(c1).astype(BF16)
        sh[f"wih_{d}"] = np.ascontiguousarray(
            np.asarray(inp[f"w_ih_{d}"], np.float32).T
        ).astype(BF16)
        sh[f"whh_{d}"] = np.ascontiguousarray(
            np.asarray(inp[f"w_hh_{d}"], np.float32).T
        ).astype(BF16)
        bias = np.asarray(inp[f"b_ih_{d}"], np.float32) + np.asarray(
            inp[f"b_hh_{d}"], np.float32
        )
        sh[f"bias_{d}"] = np.ascontiguousarray(bias.reshape(4, 128).T)  # [128,4]
    fcwT = np.asarray(inp["fc_w"], np.float32).T
    sh["fcw"] = np.ascontiguousarray(
        fcwT.reshape(4, 128, DEC).transpose(1, 0, 2).reshape(128, 4 * DEC)
    ).astype(BF16)
    outw = np.concatenate(
        [np.asarray(inp["out_w"], np.float32).T,
         np.asarray(inp["out_b"], np.float32)[None, :]], axis=0
    )  # [321, 10000]
    outwP = np.zeros((128, 3 * OUT_D), np.float32)
    for c in range(3):
        rows = outw[c * 128 : min((c + 1) * 128, 321)]
        outwP[: len(rows), c * OUT_D : (c + 1) * OUT_D] = rows
    sh["outw"] = outwP.astype(BF16)
    return sh


# ---- bass program ---------------------------------------------------------

def build_nc(meta, shapes):
    import concourse.bass as bass
    import concourse.bacc as bacc
    import concourse.tile as tile
    import concourse.mybir as mybir
    from concourse.library_config import mlp as mlp_lib

    dt = mybir.dt
    AF = mybir.ActivationFunctionType
    ALU = mybir.AluOpType
    nb = meta["nb"]
    NP = meta["np"]
    NT128 = NP // 128
    SPANS = [(n, min(1024, NP - n)) for n in range(0, NP, 1024)]

    nc = bacc.Bacc("TRN2", target_bir_lowering=False, debug=False,
                   num_devices=N_CORES)

    dr = {}
    for name, arr_shape, dtype in shapes:
        dr[name] = nc.dram_tensor(name, list(arr_shape), dtype,
                                  kind="ExternalInput")
    out_d = nc.dram_tensor("out", [SPC, OUT_D], dt.float32, kind="ExternalOutput")

    boff = {}
    for key, arr in nb.items():
        boff[key] = np.concatenate([[0], np.cumsum(arr)])

    with tile.TileContext(nc) as tc, ExitStack() as ctx:
        nc.gpsimd.load_library(mlp_lib)

        cp = ctx.enter_context(tc.tile_pool(name="const", bufs=1))

        def load_const(name):
            h = dr[name]
            t = cp.tile(list(h.shape), h.dtype, tag=name)
            nc.sync.dma_start(t[:], h.ap()[:, :])
            return t

        # load order = DMA queue order: unblock gathers + table build first
        w = {}
        for d in ("f", "b"):
            w[f"wih_{d}"] = load_const(f"wih_{d}")
            w[f"whh_{d}"] = load_const(f"whh_{d}")
            w[f"bias_{d}"] = load_const(f"bias_{d}")
        ramp = load_const("ramp")
        v0idx = load_const("v0idx")
        v8idx = load_const("v8idx")
        ctxid = {}
        sidx = {}
        for side in ("ll", "rl"):
            for cls in ("lo", "hi"):
                ctxid[(side, cls)] = load_const(f"{side}_{cls}_ctx")
                sidx[(side, cls)] = load_const(f"{side}_{cls}_idx")
        xidx = load_const("xidx")
        a_bc = load_const("a_bc")
        fcw = load_const("fcw")
        sampid = load_const("sampid")

        big = ctx.enter_context(tc.tile_pool(name="big", bufs=1))
        embT = {s: big.tile([128, NP], dt.bfloat16, tag=f"embT_{s}",
                            name=f"embT_{s}") for s in ("ll", "rl")}
        hT = {d: big.tile([128, NP], dt.bfloat16, tag=f"hT_{d}",
                          name=f"hT_{d}") for d in ("f", "b")}

        # ---- pools -----------------------------------------------------
        # outer pools first, then span-loop pools in an inner scope that is
        # freed before the tail (stack order respected)
        csp = ctx.enter_context(tc.tile_pool(name="cstage", bufs=2))
        wp = ctx.enter_context(tc.tile_pool(name="wstream", bufs=3))
        pfp = ctx.enter_context(tc.tile_pool(name="psF", bufs=3, space="PSUM"))
        sctx = ctx.enter_context(ExitStack())
        gp = sctx.enter_context(tc.tile_pool(name="gdst", bufs=2))
        ohp = sctx.enter_context(tc.tile_pool(name="oh", bufs=2))
        xp = sctx.enter_context(tc.tile_pool(name="xt", bufs=2))
        hgp = sctx.enter_context(tc.tile_pool(name="hcg", bufs=2))
        sp = sctx.enter_context(tc.tile_pool(name="stage", bufs=2))
        hcp = sctx.enter_context(tc.tile_pool(name="hc", bufs=2))
        pap = sctx.enter_context(tc.tile_pool(name="psA", bufs=1, space="PSUM"))
        pbp = sctx.enter_context(tc.tile_pool(name="psB", bufs=2, space="PSUM"))

        # attention accumulator in SBUF (PSUM accumulation groups cannot
        # share a 2KB zero-region): cols 0:65 = v-chunk0, 65:130 = v-chunk1,
        # 130:195 = v-chunk2 (rows<65), 195:196 = sum-of-exp (rows<65)
        att = big.tile([128, 212], dt.float32, tag="att", name="att")
        nc.vector.memset(att[:], 0)

        # ================= phase A: subtoken segment sums ================
        def emit_A_gathers(grps):
            dsts = {}
            for side in ("ll", "rl"):
                tblmap = {"lo": dr["sub_lo"], "hi": dr["sub_hi"]}
                for g in grps:
                    t0, t1 = g * GROUP, min((g + 1) * GROUP, NT128)
                    dst = {}
                    for cls in ("lo", "hi"):
                        b0, b1 = boff[(side, cls)][t0], boff[(side, cls)][t1]
                        nblk = int(b1 - b0)
                        if nblk == 0:
                            continue
                        chunks = []
                        for c0 in range(0, nblk, 8):
                            cn = min(8, nblk - c0)
                            dtile = gp.tile([128, 8 * 128], dt.bfloat16,
                                            tag=f"g_{cls}", name=f"g_{cls}",
                                            bufs=3)
                            d3 = dtile[:].rearrange("p (b e) -> p b e", e=128)
                            nc.gpsimd.dma_gather(
                                d3[:, 0:cn, :],
                                tblmap[cls].ap()[:, :],
                                sidx[(side, cls)][:, (int(b0) + c0) * 8 :
                                                  (int(b0) + c0 + cn) * 8],
                                cn * 128,
                                cn * 128,
                                128,
                            )
                            chunks.append(dtile)
                        dst[cls] = (chunks, int(b0))
                    dsts[(side, g)] = dst
            return dsts

        def emit_A_compute(grps, dsts):
            for side in ("ll", "rl"):
                for g in grps:
                    t0, t1 = g * GROUP, min((g + 1) * GROUP, NT128)
                    dst = dsts[(side, g)]
                    for t in range(t0, t1):
                        blocks = []
                        for cls in ("lo", "hi"):
                            nbt = int(nb[(side, cls)][t])
                            if nbt == 0:
                                continue
                            chunks, gb0 = dst[cls]
                            tb0 = int(boff[(side, cls)][t])
                            for j in range(nbt):
                                lb = tb0 - gb0 + j
                                # (chunk tile, block within chunk, ctxid col)
                                blocks.append((chunks[lb // 8], lb % 8,
                                               ctxid[(side, cls)], tb0 + j))
                        if not blocks:
                            nc.vector.memset(embT[side][:, t * 128 : (t + 1) * 128], 0)
                            continue
                        ps = pap.tile([128, 128], dt.float32, tag="psA",
                                      name="psA")
                        for j, (dtile, lb, cid, gcol) in enumerate(blocks):
                            oh = ohp.tile([128, 128], dt.bfloat16, tag="oh",
                                          name="oh")
                            nc.vector.tensor_scalar(
                                out=oh[:], in0=ramp[:],
                                scalar1=cid[:, gcol : gcol + 1],
                                scalar2=None, op0=ALU.is_equal,
                            )
                            nc.tensor.matmul(
                                out=ps[:],
                                lhsT=dtile[:, lb * 128 : (lb + 1) * 128],
                                rhs=oh[:],
                                start=(j == 0),
                                stop=(j == len(blocks) - 1),
                            )
                        nc.vector.tensor_copy(
                            out=embT[side][:, t * 128 : (t + 1) * 128], in_=ps[:]
                        )

        # ================= phase B: bidirectional LSTM ===================
        def emit_gathers(n0, W):
            """x + h1/c1 gathers for a span; returns (xt, hp, cp_)."""
            xt = xp.tile([128, T * 1024], dt.bfloat16, tag="xt", name="xt")
            x3 = xt[:].rearrange("p (a n) -> p a n", a=1)
            hp, cp_ = {}, {}
            if T0_TABLE:
                for d, vids in (("f", v0idx), ("b", v8idx)):
                    for hc_, store in (("h", hp), ("c", cp_)):
                        t_ = hgp.tile([128, 1024], dt.bfloat16,
                                      tag=f"hc_{d}{hc_}", name=f"hc_{d}{hc_}")
                        t3 = t_[:].rearrange("p (a n) -> p a n", a=1)
                        for h2 in range(0, W, 512):
                            cw = min(512, W - h2)
                            nc.gpsimd.dma_gather(
                                t3[:, :, h2 : h2 + cw],
                                dr[f"tbl_{d}{hc_}"].ap()[:, :],
                                vids[:, (n0 + h2) // 16 : (n0 + h2 + cw) // 16],
                                cw, cw, H, transpose=True,
                            )
                        store[d] = t_[:, 0:W]
            # x gathered in LSTM consumption order (t=1 needs tt=1 and 7, ...)
            for tg_ in (1, 7, 2, 6, 3, 5, 4, 8, 0):
                for h2 in range(0, W, 512):
                    cw = min(512, W - h2)
                    nc.gpsimd.dma_gather(
                        x3[:, :, tg_ * W + h2 : tg_ * W + h2 + cw],
                        dr["node_t"].ap()[:, :],
                        xidx[:, (n0 * T + tg_ * W + h2) // 16 :
                             (n0 * T + tg_ * W + h2 + cw) // 16],
                        cw, cw, 128, transpose=True,
                    )
            return xt, hp, cp_

        def emit_B(n0, W, gathered):
            xt, hp, cp_ = gathered
            if not T0_TABLE:
                # debug fallback: plain t=0 step (no f-gate, x-only matmuls)
                for d in ("f", "b"):
                    tt = 0 if d == "f" else (T - 1)
                    bias = w[f"bias_{d}"]
                    pgs = {}
                    for gi in (0, 2, 3):
                        pg = pbp.tile([128, 1024], dt.float32, tag="psB",
                                      name="psB")
                        for h2 in range(0, W, 512):
                            cw = min(512, W - h2)
                            nc.tensor.matmul(
                                out=pg[:, h2 : h2 + cw],
                                lhsT=w[f"wih_{d}"][:, gi * 128 : (gi + 1) * 128],
                                rhs=xt[:, tt * W + h2 : tt * W + h2 + cw],
                                start=True, stop=True,
                            )
                        pgs[gi] = pg
                    si = sp.tile([128, 1024], dt.bfloat16, tag="si", name="si")
                    nc.scalar.activation(si[:, 0:W], pgs[0][:, 0:W], AF.Sigmoid,
                                         bias=bias[:, 0:1])
                    tg = sp.tile([128, 1024], dt.bfloat16, tag="tg", name="tg")
                    nc.scalar.activation(tg[:, 0:W], pgs[2][:, 0:W], AF.Tanh,
                                         bias=bias[:, 2:3])
                    so = sp.tile([128, 1024], dt.bfloat16, tag="so", name="so")
                    nc.scalar.activation(so[:, 0:W], pgs[3][:, 0:W], AF.Sigmoid,
                                         bias=bias[:, 3:4])
                    c_new = hcp.tile([128, 1024], dt.bfloat16, tag="c", name="c")
                    nc.vector.tensor_tensor(out=c_new[:, 0:W], in0=si[:, 0:W],
                                            in1=tg[:, 0:W], op=ALU.mult)
                    nc.scalar.activation(tg[:, 0:W], c_new[:, 0:W], AF.Tanh)
                    h_new = hcp.tile([128, 1024], dt.bfloat16, tag="h", name="h")
                    nc.vector.tensor_tensor(out=h_new[:, 0:W], in0=so[:, 0:W],
                                            in1=tg[:, 0:W], op=ALU.mult)
                    hp[d], cp_[d] = h_new[:, 0:W], c_new[:, 0:W]

            for t in range(1, T):
                for d in ("f", "b"):
                    h_prev = hp[d]
                    c_prev = cp_[d]
                    tt = t if d == "f" else (T - 1 - t)
                    bias = w[f"bias_{d}"]
                    psg = []
                    for gi in range(4):
                        pg = pbp.tile([128, 1024], dt.float32, tag="psB",
                                      name="psB")
                        for h2 in range(0, W, 512):
                            cw = min(512, W - h2)
                            nc.tensor.matmul(
                                out=pg[:, h2 : h2 + cw],
                                lhsT=w[f"wih_{d}"][:, gi * 128 : (gi + 1) * 128],
                                rhs=xt[:, tt * W + h2 : tt * W + h2 + cw],
                                start=True, stop=False,
                            )
                            nc.tensor.matmul(
                                out=pg[:, h2 : h2 + cw],
                                lhsT=w[f"whh_{d}"][:, gi * 128 : (gi + 1) * 128],
                                rhs=h_prev[:, h2 : h2 + cw],
                                start=False, stop=True,
                            )
                        psg.append(pg)
                    si = sp.tile([128, 1024], dt.bfloat16, tag="si", name="si")
                    nc.scalar.activation(si[:, 0:W], psg[0][:, 0:W], AF.Sigmoid,
                                         bias=bias[:, 0:1])
                    sf = sp.tile([128, 1024], dt.bfloat16, tag="sf", name="sf")
                    nc.scalar.activation(sf[:, 0:W], psg[1][:, 0:W], AF.Sigmoid,
                                         bias=bias[:, 1:2])
                    tg = sp.tile([128, 1024], dt.bfloat16, tag="tg", name="tg")
                    nc.scalar.activation(tg[:, 0:W], psg[2][:, 0:W], AF.Tanh,
                                         bias=bias[:, 2:3])
                    so = sp.tile([128, 1024], dt.bfloat16, tag="so", name="so")
                    nc.scalar.activation(so[:, 0:W], psg[3][:, 0:W], AF.Sigmoid,
                                         bias=bias[:, 3:4])
                    # in-place: u = si*tg -> si ; v = sf*c -> sf ; th -> tg
                    nc.vector.tensor_tensor(out=si[:, 0:W], in0=si[:, 0:W],
                                            in1=tg[:, 0:W], op=ALU.mult)
                    nc.vector.tensor_tensor(out=sf[:, 0:W], in0=sf[:, 0:W],
                                            in1=c_prev[:, 0:W], op=ALU.mult)
                    c_new = hcp.tile([128, 1024], dt.bfloat16, tag="c", name="c")
                    nc.vector.tensor_tensor(out=c_new[:, 0:W], in0=si[:, 0:W],
                                            in1=sf[:, 0:W], op=ALU.add)
                    nc.scalar.activation(tg[:, 0:W], c_new[:, 0:W], AF.Tanh)
                    if t == T - 1:
                        h_new = None
                        nc.vector.tensor_tensor(
                            out=hT[d][:, n0 : n0 + W], in0=so[:, 0:W],
                            in1=tg[:, 0:W], op=ALU.mult)
                    else:
                        h_new = hcp.tile([128, 1024], dt.bfloat16, tag="h",
                                         name="h")
                        nc.vector.tensor_tensor(
                            out=h_new[:, 0:W], in0=so[:, 0:W],
                            in1=tg[:, 0:W], op=ALU.mult)
                    hp[d], cp_[d] = (h_new[:, 0:W] if h_new is not None else None,
                                     c_new[:, 0:W])

        # ====== phase C: fc + attention (per span, accumulates att) ======
        def emit_C(tiles128):
            feats = [embT["ll"], hT["f"], hT["b"], embT["rl"]]
            nts = len(tiles128)
            cxts = []
            score = csp.tile([128, 16], dt.float32, tag="score", name="score")
            for ti, t in enumerate(tiles128):
                sl = slice(t * 128, (t + 1) * 128)
                psf = pfp.tile([128, 512], dt.float32, tag="psf", name="psf")
                for gi in range(4):
                    nc.tensor.matmul(
                        out=psf[:, 0:DEC],
                        lhsT=feats[gi][:, sl],
                        rhs=fcw[:, gi * DEC : (gi + 1) * DEC],
                        start=(gi == 0), stop=(gi == 3),
                    )
                cxt = csp.tile([128, 322], dt.bfloat16, tag="cxt", name="cxt",
                               bufs=4)
                nc.scalar.activation(cxt[:, 0:DEC], psf[:, 0:DEC], AF.Tanh)
                nc.vector.memset(cxt[:, DEC : DEC + 2], 1.0)
                prod = csp.tile([128, 322], dt.bfloat16, tag="ectx", name="prod")
                nc.vector.tensor_tensor(out=prod[:, 0:DEC], in0=cxt[:, 0:DEC],
                                        in1=a_bc[:], op=ALU.mult)
                nc.vector.tensor_reduce(out=score[:, ti : ti + 1], in_=prod[:, 0:DEC],
                                        axis=mybir.AxisListType.X, op=ALU.add)
                cxts.append(cxt)
            # exp(score) via tanh identity, batched over the span's tiles:
            # th = tanh(score/2); e = (1+th)/(1-th)
            sth = score
            nc.scalar.activation(sth[:, 0:nts], score[:, 0:nts], AF.Tanh,
                                 scale=0.5)
            den = csp.tile([128, 16], dt.float32, tag="den", name="den")
            nc.vector.tensor_scalar(out=den[:, 0:nts], in0=sth[:, 0:nts],
                                    scalar1=-1.0, scalar2=1.0,
                                    op0=ALU.mult, op1=ALU.add)
            rden = csp.tile([128, 16], dt.float32, tag="rden", name="rden")
            nc.vector.reciprocal(rden[:, 0:nts], den[:, 0:nts])
            ee = csp.tile([128, 16], dt.float32, tag="ee", name="ee")
            nc.vector.scalar_tensor_tensor(
                out=ee[:, 0:nts], in0=sth[:, 0:nts], scalar=1.0,
                in1=rden[:, 0:nts], op0=ALU.add, op1=ALU.mult)
            for ti, t in enumerate(tiles128):
                cxt = cxts[ti]
                ectx = csp.tile([128, 322], dt.bfloat16, tag="ectx", name="ectx")
                nc.vector.tensor_scalar(
                    out=ectx[:, 0:322], in0=cxt[:, 0:322],
                    scalar1=ee[:, ti : ti + 1], scalar2=None, op0=ALU.mult,
                )
                ohb = csp.tile([128, 65], dt.bfloat16, tag="ohb", name="ohb")
                nc.vector.tensor_scalar(
                    out=ohb[:], in0=ramp[:, 0:65],
                    scalar1=sampid[:, t : t + 1], scalar2=None,
                    op0=ALU.is_equal,
                )
                patt = pfp.tile([128, 512], dt.float32, tag="psf", name="psf")
                nc.tensor.matmul(out=patt[:, 0:65], lhsT=ectx[:, 0:128],
                                 rhs=ohb[:], start=True, stop=True)
                nc.tensor.matmul(out=patt[:, 65:130], lhsT=ectx[:, 128:256],
                                 rhs=ohb[:], start=True, stop=True)
                nc.tensor.matmul(out=patt[0:65, 130:195], lhsT=ectx[:, 256:321],
                                 rhs=ohb[:], start=True, stop=True)
                nc.tensor.matmul(out=patt[0:65, 195:196], lhsT=ohb[:],
                                 rhs=ectx[:, 320:321], start=True, stop=True)
                nc.vector.tensor_tensor(out=att[:, 0:130], in0=att[:, 0:130],
                                        in1=patt[:, 0:130], op=ALU.add)
                nc.vector.tensor_tensor(out=att[0:65, 130:196],
                                        in0=att[0:65, 130:196],
                                        in1=patt[0:65, 130:196], op=ALU.add)

        # ================= main loop =====================================
        def emit_compute(n0, W, g, dsts):
            grps = range(n0 // 512, (n0 + W + 511) // 512)
            emit_B(n0, W, g)
            emit_A_compute(grps, dsts)
            for c0 in range(n0 // 128, (n0 + W) // 128, 3):
                emit_C(range(c0, min(c0 + 3, (n0 + W) // 128)))

        # issue span s+1's gathers ahead of span s's compute so the Pool
        # queue keeps the LSTM fed across span boundaries
        pend = None
        OW = 1000
        rts = []
        ow3 = dr["outw"].ap()[:, :].rearrange("p (c n) -> p c n", c=3)
        for j in range(OUT_D // OW):
            rt = wp.tile([128, 3 * OW], dt.bfloat16, tag="rt", name="rt",
                         bufs=2)
            r3 = rt[:].rearrange("p (c n) -> p c n", c=3)
            nc.sync.dma_start(r3[:, :, :], ow3[:, :, j * OW : (j + 1) * OW])
            rts.append(rt)
        for si_, (n0, W) in enumerate(SPANS):
            g = emit_gathers(n0, W)
            dsts = emit_A_gathers(range(n0 // 512, (n0 + W + 511) // 512))
            if pend is not None:
                emit_compute(*pend)
            pend = (n0, W, g, dsts)
        emit_compute(*pend)

        # ================= finalize: v, S, output matmul =================
        sctx.close()   # free span-loop pools for the outw stream
        vt = [csp.tile([128, 65], dt.bfloat16, tag=f"vt{c}", name=f"vt{c}")
              for c in range(2)]
        vt2 = csp.tile([65, 65], dt.bfloat16, tag="vt2", name="vt2")
        nc.vector.tensor_copy(out=vt[0][:], in_=att[:, 0:65])
        nc.vector.tensor_copy(out=vt[1][:], in_=att[:, 65:130])
        nc.vector.tensor_copy(out=vt2[:, :], in_=att[0:65, 130:195])
        sinv = csp.tile([SPC, 16], dt.float32, tag="sinv", name="sinv")
        nc.vector.reciprocal(sinv[:, 0:1], att[0:SPC, 195:196])

        with tc.tile_pool(name="otp", bufs=3) as otp:
            for j in range(OUT_D // OW):
                rt = rts[j]
                ot = otp.tile([SPC, OW], dt.float32, tag="ot", name="ot")
                for s5 in range(OW // 500):
                    c0 = s5 * 500
                    pot = pfp.tile([128, 512], dt.float32, tag="psf",
                                   name="psf")
                    po = pot[0:SPC, 0:500]
                    nc.tensor.matmul(out=po, lhsT=vt[0][:, 0:SPC],
                                     rhs=rt[:, c0 : c0 + 500],
                                     start=True, stop=False)
                    nc.tensor.matmul(out=po, lhsT=vt[1][:, 0:SPC],
                                     rhs=rt[:, OW + c0 : OW + c0 + 500],
                                     start=False, stop=False)
                    nc.tensor.matmul(out=po, lhsT=vt2[:, 0:SPC],
                                     rhs=rt[0:65, 2 * OW + c0 : 2 * OW + c0 + 500],
                                     start=False, stop=True)
                    nc.scalar.activation(ot[:, c0 : c0 + 500], po, AF.Copy,
                                         scale=sinv[:, 0:1])
                nc.sync.dma_start(out_d.ap()[:, j * OW : (j + 1) * OW], ot[:])

    nc.compile()
    return nc


# ---- top-level entry ------------------------------------------------------

_CACHE = {}


def _build_in_maps(meta, cores, sh):
    in_maps = []
    for d in cores:
        m = {
            "sub_lo": sh["sub_lo"], "sub_hi": sh["sub_hi"],
            "node_t": sh["node_t"],
            "ramp": sh["ramp"], "a_bc": sh["a_bc"],
            "fcw": sh["fcw"], "outw": sh["outw"],
            "xidx": d["xidx"], "v0idx": d["v0idx"], "v8idx": d["v8idx"],
            "sampid": d["sampid"],
        }
        for dd in ("f", "b"):
            m[f"tbl_{dd}h"] = sh[f"tbl_{dd}h"]
            m[f"tbl_{dd}c"] = sh[f"tbl_{dd}c"]
            m[f"wih_{dd}"] = sh[f"wih_{dd}"]
            m[f"whh_{dd}"] = sh[f"whh_{dd}"]
            m[f"bias_{dd}"] = sh[f"bias_{dd}"]
        for side in ("ll", "rl"):
            for cls in ("lo", "hi"):
                m[f"{side}_{cls}_idx"] = d[f"{side}_{cls}_idx"]
                m[f"{side}_{cls}_ctx"] = d[f"{side}_{cls}_ctx"]
        in_maps.append(m)
    return in_maps


def _shapes_of(in_map):
    import concourse.mybir as mybir
    dt = mybir.dt
    np2my = {
        np.dtype(np.float32): dt.float32,
        np.dtype(BF16): dt.bfloat16,
        np.dtype(np.int16): dt.int16,
    }
    return [(k, list(v.shape), np2my[v.dtype]) for k, v in sorted(in_map.items())]


def kernel(**inputs):
    from concourse.bass_utils import run_bass_kernel_spmd

    inp = {k: np.asarray(v) for k, v in inputs.items()}
    meta, cores = prep_all(inp)
    sh = prep_shared(inp)
    in_maps = _build_in_maps(meta, cores, sh)
    shapes = _shapes_of(in_maps[0])
    key = (str(shapes) + str(meta["np"])
           + str({k: v.tolist() for k, v in meta["nb"].items()}))
    if key not in _CACHE:
        _CACHE[key] = build_nc(meta, shapes)
    nc = _CACHE[key]
    res = run_bass_kernel_spmd(nc, in_maps, core_ids=list(range(N_CORES)))
    return np.concatenate([res.results[c]["out"] for c in range(N_CORES)], axis=0)



# revision 53
# speedup vs baseline: 1.0098x; 1.0098x over previous
"""Trainium2 Bass kernel for nn_C2SModel (code2seq-style model), v2.

Optimizations over baseline:
  - t=0 LSTM step collapsed to a 512-entry vocab table (h1,c1), built on
    device, DMA-gathered per span.
  - Cell elementwise via scalar_tensor_tensor (4x DVE mode).
  - Phase C (fc + attention) pipelined into the span loop; exp computed
    via tanh identity to keep one activation table set.
  - Dynamic NP (pad to actual max contexts per core).
  - Batched gathers (1024-row calls).
"""
import os
import numpy as np
import ml_dtypes
from contextlib import ExitStack

BF16 = ml_dtypes.bfloat16
T0_TABLE = os.environ.get("NO_T0_TABLE", "") != "1"
GX = int(os.environ.get("GX", "512"))  # gather rows per call (1024 crashes HW)
HCMERGE = os.environ.get("USE_HCMERGE", "") == "1"  # needs 1024-row gather: crashes
ATT_PSUM = os.environ.get("NO_ATT_PSUM", "") != "1"
TP64 = os.environ.get("NO_TP64", "") != "1"     # 128-part out chunks
XSTREAM = os.environ.get("NO_XSTREAM", "") != "1"
TTR = os.environ.get("USE_TTR", "") == "1"  # InstTensorTensorReduce crashes on HW
ILV = os.environ.get("NO_ILV", "") != "1"       # interleaved emission

# ---- problem constants (hardcoded per contract) ---------------------------
N_CTX = 65536
B = 512
E = 128
H = 128
T = 9
SUB_V = 50000
NODE_V = 512
DEC = 320
OUT_D = 10000
N_CORES = 8
SPC = B // N_CORES            # 64 samples per core
SPLIT = 32768                 # subtoken table split (int16 gather indices)
GROUP = 4                     # ctx-tiles of 128 per gather call group
OUT_NCH = 500                 # final matmul N-chunk (20 chunks of 500)


# ---- host-side prep -------------------------------------------------------

def _wrap_idx(ids):
    ids = np.asarray(ids, np.int16)
    assert len(ids) % 16 == 0
    w16 = ids.reshape(-1, 16).T
    return np.tile(w16, (8, 1)).copy()


def _pad_to(x, n, val):
    out = np.full((n,) + x.shape[1:], val, x.dtype)
    out[: len(x)] = x
    return out


def _core_raw(k, inp, NP):
    """Per-core raw occurrence lists and context data."""
    NT128 = NP // 128
    indices = inp["indices"]
    s = int(np.searchsorted(indices, k * SPC, "left"))
    e = int(np.searchsorted(indices, (k + 1) * SPC, "left"))
    nk = e - s
    assert nk <= NP, f"core {k}: {nk} contexts exceed NP={NP}"
    samp = _pad_to(indices[s:e].astype(np.int32) - k * SPC, NP, -1)
    pth = _pad_to(inp["paths"][s:e].astype(np.int16), NP, 0)
    d = {"samp": samp, "paths": pth, "nk": nk}
    for side in ("ll", "rl"):
        idxs_g = inp[f"{side}_indices"]
        subs_g = inp[f"{side}_subtokens"]
        o_s = int(np.searchsorted(idxs_g, s, "left"))
        o_e = int(np.searchsorted(idxs_g, e, "left"))
        subs = subs_g[o_s:o_e].astype(np.int32)
        ctxs = idxs_g[o_s:o_e].astype(np.int32) - s
        tb = np.searchsorted(ctxs, np.arange(0, NP + 128, 128))
        per_tile = {"lo": [], "hi": []}
        for t in range(NT128):
            sl = slice(tb[t], tb[t + 1])
            tsub, tctx = subs[sl], ctxs[sl] - t * 128
            m = tsub < SPLIT
            per_tile["lo"].append((tsub[m], tctx[m]))
            per_tile["hi"].append((tsub[~m] - SPLIT, tctx[~m]))
        d[side] = per_tile
    return d


def prep_all(inp):
    """Returns (meta, per_core_data)."""
    indices = inp["indices"]
    counts = [
        int(np.searchsorted(indices, (k + 1) * SPC, "left"))
        - int(np.searchsorted(indices, k * SPC, "left"))
        for k in range(N_CORES)
    ]
    NP = ((max(counts) + 127) // 128) * 128   # pad to 128 multiple
    NT128 = NP // 128
    raws = [_core_raw(k, inp, NP) for k in range(N_CORES)]
    nb = {}
    for side in ("ll", "rl"):
        for cls in ("lo", "hi"):
            nb[(side, cls)] = np.array(
                [
                    max((len(r[side][cls][t][0]) + 127) // 128 for r in raws)
                    for t in range(NT128)
                ],
                np.int32,
            )
    meta = {"nb": nb, "np": NP}
    SPANS = [(n, min(1024, NP - n)) for n in range(0, NP, 1024)]

    cores = []
    for r in raws:
        d = {"samp": r["samp"], "nk": r["nk"]}
        pth = r["paths"]
        xidx = np.concatenate(
            [pth[n0 : n0 + W].T.reshape(-1) for (n0, W) in SPANS]
        )
        d["xidx"] = _wrap_idx(xidx)
        d["v0idx"] = _wrap_idx(pth[:, 0].copy())
        d["v8idx"] = _wrap_idx(pth[:, T - 1].copy())
        d["sampid"] = np.ascontiguousarray(
            r["samp"].reshape(NT128, 128).T.astype(np.float32)
        )
        for side in ("ll", "rl"):
            for cls in ("lo", "hi"):
                subs_s, ctxs_s = [], []
                for t in range(NT128):
                    ts_, tc_ = r[side][cls][t]
                    n = nb[(side, cls)][t] * 128
                    subs_s.append(_pad_to(ts_.astype(np.int32), n, 0))
                    ctxs_s.append(_pad_to(tc_.astype(np.int32), n, -1))
                subs_s = np.concatenate(subs_s) if subs_s else np.zeros(0, np.int32)
                ctxs_s = np.concatenate(ctxs_s) if ctxs_s else np.zeros(0, np.int32)
                d[f"{side}_{cls}_idx"] = _wrap_idx(subs_s.astype(np.int16))
                d[f"{side}_{cls}_ctx"] = np.ascontiguousarray(
                    ctxs_s.reshape(-1, 128).T.astype(np.float32)
                )
        cores.append(d)
    return meta, cores


def prep_shared(inp):
    """Replicated (same for all cores) tensors."""
    sub = np.asarray(inp["subtoken_emb"], np.float32)
    node = np.asarray(inp["node_emb"], np.float32)
    sh = {
        "sub_lo": sub[:SPLIT].astype(BF16),
        "sub_hi": sub[SPLIT:].astype(BF16),
        "node_t": node.astype(BF16),
        "nodeT": np.ascontiguousarray(node.T).astype(BF16),  # [128 e, 512 v]
        "ramp": np.tile(np.arange(128, dtype=np.float32), (128, 1)).astype(BF16),
        "iotap": np.tile(
            np.arange(128, dtype=np.float32)[:, None], (1, 8)
        ),  # [128, 8] f32, col j = partition idx
        "a_bc": np.tile(np.asarray(inp["a"], np.float32), (128, 1)).astype(BF16),
        # [64,128] fp32: iddup[k,m] = (k == m%64); broadcasts the per-sample
        # sum-of-exp column to 128 partitions via one tiny fp32 matmul
        "iddup": (np.arange(64)[:, None] == (np.arange(128)[None, :] % 64))
        .astype(np.float32),
    }
    def _sig(x):
        return 1.0 / (1.0 + np.exp(-x))

    for d in ("f", "b"):
        # t=0 LSTM step for all 512 path-node vocab entries (h=c=0):
        # c1 = sig(i)*tanh(g); h1 = sig(o)*tanh(c1)   (gate order i,f,g,o)
        wih = np.asarray(inp[f"w_ih_{d}"], np.float32)
        bias0 = (np.asarray(inp[f"b_ih_{d}"], np.float32)
                 + np.asarray(inp[f"b_hh_{d}"], np.float32))
        g0 = node @ wih.T + bias0                        # [512, 512]
        c1 = _sig(g0[:, 0:128]) * np.tanh(g0[:, 256:384])
        h1 = _sig(g0[:, 384:512]) * np.tanh(c1)
        # combined h|c table: one 256-elem gather fetches both states
        sh[f"tbl_{d}"] = np.ascontiguousarray(
            np.concatenate([h1, c1], axis=1)
        ).astype(BF16)
        sh[f"tbl_{d}h"] = np.ascontiguousarray(h1).astype(BF16)
        sh[f"tbl_{d}c"] = np.ascontiguousarray(c1).astype(BF16)
        sh[f"wih_{d}"] = np.ascontiguousarray(
            np.asarray(inp[f"w_ih_{d}"], np.float32).T
        ).astype(BF16)
        sh[f"whh_{d}"] = np.ascontiguousarray(
            np.asarray(inp[f"w_hh_{d}"], np.float32).T
        ).astype(BF16)
        bias = np.asarray(inp[f"b_ih_{d}"], np.float32) + np.asarray(
            inp[f"b_hh_{d}"], np.float32
        )
        sh[f"bias_{d}"] = np.ascontiguousarray(bias.reshape(4, 128).T)  # [128,4]
    fcwT = np.asarray(inp["fc_w"], np.float32).T
    sh["fcw"] = np.ascontiguousarray(
        fcwT.reshape(4, 128, DEC).transpose(1, 0, 2).reshape(128, 4 * DEC)
    ).astype(BF16)
    outw = np.concatenate(
        [np.asarray(inp["out_w"], np.float32).T,
         np.asarray(inp["out_b"], np.float32)[None, :]], axis=0
    )  # [321, 10000]
    outwP = np.zeros((128, 3 * OUT_D), np.float32)
    for c in range(3):
        rows = outw[c * 128 : min((c + 1) * 128, 321)]
        outwP[: len(rows), c * OUT_D : (c + 1) * OUT_D] = rows
    sh["outw"] = outwP.astype(BF16)
    return sh


# ---- bass program ---------------------------------------------------------

def build_nc(meta, shapes):
    import concourse.bass as bass
    import concourse.bacc as bacc
    import concourse.tile as tile
    import concourse.mybir as mybir
    from concourse.library_config import mlp as mlp_lib

    dt = mybir.dt
    AF = mybir.ActivationFunctionType
    ALU = mybir.AluOpType
    nb = meta["nb"]
    NP = meta["np"]
    NT128 = NP // 128
    SPANS = [(n, min(1024, NP - n)) for n in range(0, NP, 1024)]

    nc = bacc.Bacc("TRN2", target_bir_lowering=False, debug=False,
                   num_devices=N_CORES,
                   dynamic_dma_scratch_size=32768)

    dr = {}
    for name, arr_shape, dtype in shapes:
        dr[name] = nc.dram_tensor(name, list(arr_shape), dtype,
                                  kind="ExternalInput")
    out_d = nc.dram_tensor("out", [SPC, OUT_D], dt.float32, kind="ExternalOutput")

    boff = {}
    for key, arr in nb.items():
        boff[key] = np.concatenate([[0], np.cumsum(arr)])

    with tile.TileContext(nc) as tc, ExitStack() as ctx:
        nc.gpsimd.load_library(mlp_lib)

        cp = ctx.enter_context(tc.tile_pool(name="const", bufs=1))

        def load_const(name):
            h = dr[name]
            t = cp.tile(list(h.shape), h.dtype, tag=name)
            nc.sync.dma_start(t[:], h.ap()[:, :])
            return t

        # load order = DMA queue order: unblock gathers + table build first
        w = {}
        for d in ("f", "b"):
            w[f"wih_{d}"] = load_const(f"wih_{d}")
            w[f"whh_{d}"] = load_const(f"whh_{d}")
            w[f"bias_{d}"] = load_const(f"bias_{d}")
        ramp = load_const("ramp")
        v0idx = load_const("v0idx")
        v8idx = load_const("v8idx")
        ctxid = {}
        sidx = {}
        a_bc = fcw = sampid = iddup = None
        xidx_full = [None]
        if not XSTREAM:
            xidx_full[0] = load_const("xidx")

        def load_late():
            # phase A/C constants aren't needed until mid-span-0; loading
            # them after span-0's gather stream keeps HWDGE off the LSTM's
            # critical path at startup
            nonlocal a_bc, fcw, sampid, iddup
            for side in ("ll", "rl"):
                for cls in ("lo", "hi"):
                    ctxid[(side, cls)] = load_const(f"{side}_{cls}_ctx")
                    sidx[(side, cls)] = load_const(f"{side}_{cls}_idx")
            a_bc = load_const("a_bc")
            fcw = load_const("fcw")
            sampid = load_const("sampid")
            iddup = load_const("iddup")

        big = ctx.enter_context(tc.tile_pool(name="big", bufs=1))
        embT = {s: big.tile([128, NP], dt.bfloat16, tag=f"embT_{s}",
                            name=f"embT_{s}") for s in ("ll", "rl")}
        hT = {d: big.tile([128, NP], dt.bfloat16, tag=f"hT_{d}",
                          name=f"hT_{d}") for d in ("f", "b")}

        # ---- pools -----------------------------------------------------
        # outer pools first, then span-loop pools in an inner scope that is
        # freed before the tail (stack order respected)
        csp = ctx.enter_context(tc.tile_pool(name="cstage", bufs=2))
        wp = ctx.enter_context(tc.tile_pool(name="wstream", bufs=3))
        pfp = ctx.enter_context(tc.tile_pool(name="psF", bufs=2, space="PSUM"))
        atp = ctx.enter_context(tc.tile_pool(name="psAtt", bufs=1, space="PSUM"))
        sctx = ctx.enter_context(ExitStack())
        gp = sctx.enter_context(tc.tile_pool(name="gdst", bufs=2))
        ohp = sctx.enter_context(tc.tile_pool(name="oh", bufs=2))
        xp = sctx.enter_context(tc.tile_pool(name="xt", bufs=2))
        hgp = sctx.enter_context(tc.tile_pool(name="hcg", bufs=2))
        sp = sctx.enter_context(tc.tile_pool(name="stage", bufs=2))
        hcp = sctx.enter_context(tc.tile_pool(name="hc", bufs=2))
        pap = sctx.enter_context(tc.tile_pool(name="psA", bufs=1, space="PSUM"))
        pbp = sctx.enter_context(tc.tile_pool(name="psB", bufs=2, space="PSUM"))

        # attention accumulator in a dedicated PSUM bank, accumulated
        # directly by the per-tile matmuls (start=False onto a memset bank):
        # cols 0:65 = v-chunk0, 65:130 = v-chunk1, 130:195 = v-chunk2
        # (rows<65), 195:196 = sum-of-exp (rows<65)
        if ATT_PSUM:
            att = atp.tile([128, 512], dt.float32, tag="att", name="att")
            nc.vector.memset(att[:, 0:256], 0)
        else:
            att = big.tile([128, 212], dt.float32, tag="att", name="att")
            nc.vector.memset(att[:], 0)

        # ================= phase A: subtoken segment sums ================
        def emit_A_gathers(grps):
            dsts = {}
            for side in ("ll", "rl"):
                tblmap = {"lo": dr["sub_lo"], "hi": dr["sub_hi"]}
                for g in grps:
                    t0, t1 = g * GROUP, min((g + 1) * GROUP, NT128)
                    dst = {}
                    for cls in ("lo", "hi"):
                        b0, b1 = boff[(side, cls)][t0], boff[(side, cls)][t1]
                        nblk = int(b1 - b0)
                        if nblk == 0:
                            continue
                        chunks = []
                        for c0 in range(0, nblk, 8):
                            cn = min(8, nblk - c0)
                            dtile = gp.tile([128, 8 * 128], dt.bfloat16,
                                            tag=f"g_{cls}", name=f"g_{cls}",
                                            bufs=3)
                            d3 = dtile[:].rearrange("p (b e) -> p b e", e=128)
                            nc.gpsimd.dma_gather(
                                d3[:, 0:cn, :],
                                tblmap[cls].ap()[:, :],
                                sidx[(side, cls)][:, (int(b0) + c0) * 8 :
                                                  (int(b0) + c0 + cn) * 8],
                                cn * 128,
                                cn * 128,
                                128,
                            )
                            chunks.append(dtile)
                        dst[cls] = (chunks, int(b0))
                    dsts[(side, g)] = dst
            return dsts

        def emit_A_compute_one(side, g, dsts):
                    t0, t1 = g * GROUP, min((g + 1) * GROUP, NT128)
                    dst = dsts[(side, g)]
                    for t in range(t0, t1):
                        blocks = []
                        for cls in ("lo", "hi"):
                            nbt = int(nb[(side, cls)][t])
                            if nbt == 0:
                                continue
                            chunks, gb0 = dst[cls]
                            tb0 = int(boff[(side, cls)][t])
                            for j in range(nbt):
                                lb = tb0 - gb0 + j
                                # (chunk tile, block within chunk, ctxid col)
                                blocks.append((chunks[lb // 8], lb % 8,
                                               ctxid[(side, cls)], tb0 + j))
                        if not blocks:
                            nc.vector.memset(embT[side][:, t * 128 : (t + 1) * 128], 0)
                            continue
                        ps = pap.tile([128, 128], dt.float32, tag="psA",
                                      name="psA")
                        for j, (dtile, lb, cid, gcol) in enumerate(blocks):
                            oh = ohp.tile([128, 128], dt.bfloat16, tag="oh",
                                          name="oh")
                            nc.vector.tensor_scalar(
                                out=oh[:], in0=ramp[:],
                                scalar1=cid[:, gcol : gcol + 1],
                                scalar2=None, op0=ALU.is_equal,
                            )
                            nc.tensor.matmul(
                                out=ps[:],
                                lhsT=dtile[:, lb * 128 : (lb + 1) * 128],
                                rhs=oh[:],
                                start=(j == 0),
                                stop=(j == len(blocks) - 1),
                            )
                        nc.vector.tensor_copy(
                            out=embT[side][:, t * 128 : (t + 1) * 128], in_=ps[:]
                        )

        def emit_A_tasks(grps, dsts):
            return [
                (lambda side=side, g=g: emit_A_compute_one(side, g, dsts))
                for side in ("ll", "rl")
                for g in grps
            ]

        # ================= phase B: bidirectional LSTM ===================
        def emit_gathers(n0, W):
            """x + h1/c1 gathers for a span; returns (xt, hp, cp_)."""
            # stream this span's gather indices (a small DMA) instead of
            # front-loading the whole 1.2MB xidx table before span 0
            if XSTREAM:
                xi = wp.tile([128, (1024 * T) // 16], dt.int16, tag="xidx",
                             name="xis", bufs=2)
                nc.sync.dma_start(
                    xi[:, 0 : (W * T) // 16],
                    dr["xidx"].ap()[:, (n0 * T) // 16 : ((n0 + W) * T) // 16])
                xoff = 0
            else:
                xi = xidx_full[0]
                xoff = (n0 * T) // 16
            xt = xp.tile([128, T * 1024], dt.bfloat16, tag="xt", name="xt")
            x3 = xt[:].rearrange("p (a n) -> p a n", a=1)
            hp, cp_ = {}, {}
            if T0_TABLE:
                for d, vids in (("f", v0idx), ("b", v8idx)):
                    if HCMERGE:
                        # combined [512,256] h|c table: one gather per dir
                        t_ = hgp.tile([128, 2 * W], dt.bfloat16,
                                      tag=f"hc_{d}", name=f"hc_{d}")
                        t3 = t_[:].rearrange("p (a n) -> p a n", a=2)
                        nc.gpsimd.dma_gather(
                            t3[:, :, :],
                            dr[f"tbl_{d}"].ap()[:, :],
                            vids[:, n0 // 16 : (n0 + W) // 16],
                            W, W, 2 * H, transpose=True,
                        )
                        hp[d] = t_[:, 0:W]
                        cp_[d] = t_[:, W : 2 * W]
                        continue
                    for hc_, store in (("h", hp), ("c", cp_)):
                        t_ = hgp.tile([128, 1024], dt.bfloat16,
                                      tag=f"hc_{d}{hc_}", name=f"hc_{d}{hc_}")
                        t3 = t_[:].rearrange("p (a n) -> p a n", a=1)
                        for h2 in range(0, W, GX):
                            cw = min(GX, W - h2)
                            nc.gpsimd.dma_gather(
                                t3[:, :, h2 : h2 + cw],
                                dr[f"tbl_{d}{hc_}"].ap()[:, :],
                                vids[:, (n0 + h2) // 16 : (n0 + h2 + cw) // 16],
                                cw, cw, H, transpose=True,
                            )
                        store[d] = t_[:, 0:W]
            # x gathered in LSTM consumption order (t=1 needs tt=1 and 7, ...)
            for tg_ in (1, 7, 2, 6, 3, 5, 4, 8, 0):
                for h2 in range(0, W, GX):
                    cw = min(GX, W - h2)
                    nc.gpsimd.dma_gather(
                        x3[:, :, tg_ * W + h2 : tg_ * W + h2 + cw],
                        dr["node_t"].ap()[:, :],
                        xi[:, xoff + (tg_ * W + h2) // 16 :
                           xoff + (tg_ * W + h2 + cw) // 16],
                        cw, cw, 128, transpose=True,
                    )
            return xt, hp, cp_

        def emit_B_tasks(n0, W, gathered):
            xt, hp, cp_ = gathered
            tasks = []

            def t0_fallback():
                # debug fallback: plain t=0 step (no f-gate, x-only matmuls)
                for d in ("f", "b"):
                    tt = 0 if d == "f" else (T - 1)
                    bias = w[f"bias_{d}"]
                    pgs = {}
                    for gi in (0, 2, 3):
                        pg = pbp.tile([128, 1024], dt.float32, tag="psB",
                                      name="psB")
                        for h2 in range(0, W, 512):
                            cw = min(512, W - h2)
                            nc.tensor.matmul(
                                out=pg[:, h2 : h2 + cw],
                                lhsT=w[f"wih_{d}"][:, gi * 128 : (gi + 1) * 128],
                                rhs=xt[:, tt * W + h2 : tt * W + h2 + cw],
                                start=True, stop=True,
                            )
                        pgs[gi] = pg
                    si = sp.tile([128, 1024], dt.bfloat16, tag="si", name="si")
                    nc.scalar.activation(si[:, 0:W], pgs[0][:, 0:W], AF.Sigmoid,
                                         bias=bias[:, 0:1])
                    tg = sp.tile([128, 1024], dt.bfloat16, tag="tg", name="tg")
                    nc.scalar.activation(tg[:, 0:W], pgs[2][:, 0:W], AF.Tanh,
                                         bias=bias[:, 2:3])
                    so = sp.tile([128, 1024], dt.bfloat16, tag="so", name="so")
                    nc.scalar.activation(so[:, 0:W], pgs[3][:, 0:W], AF.Sigmoid,
                                         bias=bias[:, 3:4])
                    c_new = hcp.tile([128, 1024], dt.bfloat16, tag="c", name="c")
                    nc.vector.tensor_tensor(out=c_new[:, 0:W], in0=si[:, 0:W],
                                            in1=tg[:, 0:W], op=ALU.mult)
                    nc.scalar.activation(tg[:, 0:W], c_new[:, 0:W], AF.Tanh)
                    h_new = hcp.tile([128, 1024], dt.bfloat16, tag="h", name="h")
                    nc.vector.tensor_tensor(out=h_new[:, 0:W], in0=so[:, 0:W],
                                            in1=tg[:, 0:W], op=ALU.mult)
                    hp[d], cp_[d] = h_new[:, 0:W], c_new[:, 0:W]

            if not T0_TABLE:
                tasks.append(t0_fallback)

            def step(t):
                for d in ("f", "b"):
                    h_prev = hp[d]
                    c_prev = cp_[d]
                    tt = t if d == "f" else (T - 1 - t)
                    bias = w[f"bias_{d}"]
                    psg = []
                    for gi in range(4):
                        pg = pbp.tile([128, 1024], dt.float32, tag="psB",
                                      name="psB")
                        for h2 in range(0, W, 512):
                            cw = min(512, W - h2)
                            nc.tensor.matmul(
                                out=pg[:, h2 : h2 + cw],
                                lhsT=w[f"wih_{d}"][:, gi * 128 : (gi + 1) * 128],
                                rhs=xt[:, tt * W + h2 : tt * W + h2 + cw],
                                start=True, stop=False,
                            )
                            nc.tensor.matmul(
                                out=pg[:, h2 : h2 + cw],
                                lhsT=w[f"whh_{d}"][:, gi * 128 : (gi + 1) * 128],
                                rhs=h_prev[:, h2 : h2 + cw],
                                start=False, stop=True,
                            )
                        psg.append(pg)
                    si = sp.tile([128, 1024], dt.bfloat16, tag="si", name="si")
                    nc.scalar.activation(si[:, 0:W], psg[0][:, 0:W], AF.Sigmoid,
                                         bias=bias[:, 0:1])
                    sf = sp.tile([128, 1024], dt.bfloat16, tag="sf", name="sf")
                    nc.scalar.activation(sf[:, 0:W], psg[1][:, 0:W], AF.Sigmoid,
                                         bias=bias[:, 1:2])
                    tg = sp.tile([128, 1024], dt.bfloat16, tag="tg", name="tg")
                    nc.scalar.activation(tg[:, 0:W], psg[2][:, 0:W], AF.Tanh,
                                         bias=bias[:, 2:3])
                    so = sp.tile([128, 1024], dt.bfloat16, tag="so", name="so")
                    nc.scalar.activation(so[:, 0:W], psg[3][:, 0:W], AF.Sigmoid,
                                         bias=bias[:, 3:4])
                    # in-place: u = si*tg -> si ; v = sf*c -> sf ; th -> tg
                    nc.vector.tensor_tensor(out=si[:, 0:W], in0=si[:, 0:W],
                                            in1=tg[:, 0:W], op=ALU.mult)
                    nc.vector.tensor_tensor(out=sf[:, 0:W], in0=sf[:, 0:W],
                                            in1=c_prev[:, 0:W], op=ALU.mult)
                    c_new = hcp.tile([128, 1024], dt.bfloat16, tag="c", name="c")
                    nc.vector.tensor_tensor(out=c_new[:, 0:W], in0=si[:, 0:W],
                                            in1=sf[:, 0:W], op=ALU.add)
                    nc.scalar.activation(tg[:, 0:W], c_new[:, 0:W], AF.Tanh)
                    if t == T - 1:
                        h_new = None
                        nc.vector.tensor_tensor(
                            out=hT[d][:, n0 : n0 + W], in0=so[:, 0:W],
                            in1=tg[:, 0:W], op=ALU.mult)
                    else:
                        h_new = hcp.tile([128, 1024], dt.bfloat16, tag="h",
                                         name="h")
                        nc.vector.tensor_tensor(
                            out=h_new[:, 0:W], in0=so[:, 0:W],
                            in1=tg[:, 0:W], op=ALU.mult)
                    hp[d], cp_[d] = (h_new[:, 0:W] if h_new is not None else None,
                                     c_new[:, 0:W])

            for t in range(1, T):
                tasks.append(lambda t=t: step(t))
            return tasks

        # ====== phase C: fc + attention (per span, accumulates att) ======
        STOPF = {"v": False}

        def emit_C(tiles128):
            feats = [embT["ll"], hT["f"], hT["b"], embT["rl"]]
            nts = len(tiles128)
            cxts = []
            score = csp.tile([128, 16], dt.float32, tag="score", name="score")
            for ti, t in enumerate(tiles128):
                sl = slice(t * 128, (t + 1) * 128)
                psf = pfp.tile([128, 512], dt.float32, tag="psf", name="psf")
                for gi in range(4):
                    nc.tensor.matmul(
                        out=psf[:, 0:DEC],
                        lhsT=feats[gi][:, sl],
                        rhs=fcw[:, gi * DEC : (gi + 1) * DEC],
                        start=(gi == 0), stop=(gi == 3),
                    )
                cxt = csp.tile([128, 322], dt.bfloat16, tag="cxt", name="cxt",
                               bufs=4)
                nc.scalar.activation(cxt[:, 0:DEC], psf[:, 0:DEC], AF.Tanh)
                nc.vector.memset(cxt[:, DEC : DEC + 2], 1.0)
                prod = csp.tile([128, 322], dt.bfloat16, tag="ectx", name="prod")
                if TTR:
                    nc.vector.tensor_tensor_reduce(
                        out=prod[:, 0:DEC], in0=cxt[:, 0:DEC], in1=a_bc[:],
                        scale=1.0, scalar=0.0, op0=ALU.mult, op1=ALU.add,
                        accum_out=score[:, ti : ti + 1],
                    )
                else:
                    nc.vector.tensor_tensor(out=prod[:, 0:DEC],
                                            in0=cxt[:, 0:DEC],
                                            in1=a_bc[:], op=ALU.mult)
                    nc.vector.tensor_reduce(out=score[:, ti : ti + 1],
                                            in_=prod[:, 0:DEC],
                                            axis=mybir.AxisListType.X,
                                            op=ALU.add)
                cxts.append(cxt)
            # exp(score) via tanh identity, batched over the span's tiles:
            # th = tanh(score/2); e = (1+th)/(1-th)
            sth = score
            nc.scalar.activation(sth[:, 0:nts], score[:, 0:nts], AF.Tanh,
                                 scale=0.5)
            den = csp.tile([128, 16], dt.float32, tag="den", name="den")
            nc.vector.tensor_scalar(out=den[:, 0:nts], in0=sth[:, 0:nts],
                                    scalar1=-1.0, scalar2=1.0,
                                    op0=ALU.mult, op1=ALU.add)
            rden = csp.tile([128, 16], dt.float32, tag="rden", name="rden")
            nc.vector.reciprocal(rden[:, 0:nts], den[:, 0:nts])
            ee = csp.tile([128, 16], dt.float32, tag="ee", name="ee")
            nc.vector.scalar_tensor_tensor(
                out=ee[:, 0:nts], in0=sth[:, 0:nts], scalar=1.0,
                in1=rden[:, 0:nts], op0=ALU.add, op1=ALU.mult)
            for ti, t in enumerate(tiles128):
                cxt = cxts[ti]
                ectx = csp.tile([128, 322], dt.bfloat16, tag="ectx", name="ectx")
                nc.vector.tensor_scalar(
                    out=ectx[:, 0:322], in0=cxt[:, 0:322],
                    scalar1=ee[:, ti : ti + 1], scalar2=None, op0=ALU.mult,
                )
                ohb = csp.tile([128, 65], dt.bfloat16, tag="ohb", name="ohb")
                nc.vector.tensor_scalar(
                    out=ohb[:], in0=ramp[:, 0:65],
                    scalar1=sampid[:, t : t + 1], scalar2=None,
                    op0=ALU.is_equal,
                )
                if ATT_PSUM:
                    stp = STOPF["v"] and (ti == nts - 1)
                    nc.tensor.matmul(out=att[:, 0:65], lhsT=ectx[:, 0:128],
                                     rhs=ohb[:], start=False, stop=stp,
                                     skip_group_check=True)
                    nc.tensor.matmul(out=att[:, 65:130], lhsT=ectx[:, 128:256],
                                     rhs=ohb[:], start=False, stop=stp,
                                     skip_group_check=True)
                    nc.tensor.matmul(out=att[0:65, 130:195],
                                     lhsT=ectx[:, 256:321],
                                     rhs=ohb[:], start=False, stop=stp,
                                     skip_group_check=True)
                    nc.tensor.matmul(out=att[0:65, 195:196], lhsT=ohb[:],
                                     rhs=ectx[:, 320:321], start=False,
                                     stop=stp, skip_group_check=True)
                else:
                    patt = pfp.tile([128, 512], dt.float32, tag="psf",
                                    name="psf")
                    nc.tensor.matmul(out=patt[:, 0:65], lhsT=ectx[:, 0:128],
                                     rhs=ohb[:], start=True, stop=True)
                    nc.tensor.matmul(out=patt[:, 65:130], lhsT=ectx[:, 128:256],
                                     rhs=ohb[:], start=True, stop=True)
                    nc.tensor.matmul(out=patt[0:65, 130:195],
                                     lhsT=ectx[:, 256:321],
                                     rhs=ohb[:], start=True, stop=True)
                    nc.tensor.matmul(out=patt[0:65, 195:196], lhsT=ohb[:],
                                     rhs=ectx[:, 320:321], start=True,
                                     stop=True)
                    nc.vector.tensor_tensor(out=att[:, 0:130],
                                            in0=att[:, 0:130],
                                            in1=patt[:, 0:130], op=ALU.add)
                    nc.vector.tensor_tensor(out=att[0:65, 130:196],
                                            in0=att[0:65, 130:196],
                                            in1=patt[0:65, 130:196],
                                            op=ALU.add)

        # ================= main loop =====================================
        # Interleave: LSTM steps of span s carry (a) phase-C groups of span
        # s-1 and (b) phase-A compute of span s between them, so the ACT and
        # DVE queues never drain while PE refills psums.
        def emit_span(n0, W, g, dsts, extra):
            grps = range(n0 // 512, (n0 + W + 511) // 512)
            btasks = emit_B_tasks(n0, W, g)
            # round-robin A (this span) with C (previous span): A frees the
            # gather-dest tiles the NEXT span's Pool queue is waiting on,
            # C keeps the ACT queue full
            atasks = emit_A_tasks(grps, dsts)
            work = []
            for i in range(max(len(atasks), len(extra))):
                if i < len(atasks):
                    work.append(atasks[i])
                if i < len(extra):
                    work.append(extra[i])
            nt = (n0 + W) // 128
            ctasks = [
                (lambda c0=c0: emit_C(range(c0, min(c0 + 3, nt))))
                for c0 in range(n0 // 128, nt, 3)
            ]
            if not ILV:
                for bt in btasks:
                    bt()
                for w_ in work:
                    w_()
                return ctasks
            nb_ = len(btasks)
            for i, bt in enumerate(btasks):
                bt()
                lo = (i * len(work)) // nb_
                hi = ((i + 1) * len(work)) // nb_
                for w_ in work[lo:hi]:
                    w_()
            return ctasks

        # issue span s+1's gathers ahead of span s's compute so the Pool
        # queue keeps the LSTM fed across span boundaries
        pend = None
        pendC = []
        OW = 1000
        for si_, (n0, W) in enumerate(SPANS):
            g = emit_gathers(n0, W)
            if si_ == 0:
                load_late()
            dsts = emit_A_gathers(range(n0 // 512, (n0 + W + 511) // 512))
            if pend is not None:
                pendC = emit_span(*pend, pendC)
            pend = (n0, W, g, dsts)
        pendC = emit_span(*pend, pendC)
        for i, w_ in enumerate(pendC):
            if i == len(pendC) - 1:
                STOPF["v"] = True
            w_()

        # ================= finalize: v, S, output matmul =================
        sctx.close()   # free span-loop pools for the outw stream
        vt = [csp.tile([128, 65], dt.bfloat16, tag=f"vt{c}", name=f"vt{c}")
              for c in range(2)]
        vt2 = csp.tile([65, 65], dt.bfloat16, tag="vt2", name="vt2")
        nc.vector.tensor_copy(out=vt[0][:], in_=att[:, 0:65])
        nc.vector.tensor_copy(out=vt[1][:], in_=att[:, 65:130])
        nc.vector.tensor_copy(out=vt2[:, :], in_=att[0:65, 130:195])
        # broadcast per-sample sum-of-exp to all 128 partitions (the output
        # chunks use partition p = h*64 + s to halve per-partition DMA bytes)
        sinv = csp.tile([128, 16], dt.float32, tag="sinv", name="sinv")
        if TP64:
            scol = csp.tile([SPC, 1], dt.float32, tag="scol", name="scol")
            nc.vector.tensor_copy(out=scol[:], in_=att[0:SPC, 195:196])
            pS = pfp.tile([128, 512], dt.float32, tag="psf", name="psf")
            nc.tensor.matmul(out=pS[:, 0:1], lhsT=iddup[:, :], rhs=scol[:],
                             start=True, stop=True)
            nc.vector.reciprocal(sinv[:, 0:1], pS[:, 0:1])
        else:
            nc.vector.reciprocal(sinv[0:SPC, 0:1], att[0:SPC, 195:196])

        # stream outw now that the span pools are freed; out chunk j covers
        # output cols [j*1000, (j+1)*1000) as [128 = (half, sample), 500]
        ow3 = dr["outw"].ap()[:, :].rearrange("p (c n) -> p c n", c=3)
        with tc.tile_pool(name="rtp", bufs=10) as rtp, \
             tc.tile_pool(name="otp", bufs=3) as otp:
            rts = []
            for j in range(OUT_D // OW):
                rt = rtp.tile([128, 3 * OW], dt.bfloat16, tag="rt", name="rt")
                r3 = rt[:].rearrange("p (c n) -> p c n", c=3)
                nc.sync.dma_start(r3[:, :, :], ow3[:, :, j * OW : (j + 1) * OW])
                rts.append(rt)
            for j in range(OUT_D // OW):
                r3 = rts[j][:].rearrange("p (c n) -> p c n", c=3)
                if TP64:
                    ot = otp.tile([128, 500], dt.float32, tag="ot", name="ot")
                    pot = pfp.tile([128, 512], dt.float32, tag="psf",
                                   name="psf")
                    for h in (0, 1):
                        po = pot[64 * h : 64 * h + SPC, 0:500]
                        hs = slice(h * 500, h * 500 + 500)
                        nc.tensor.matmul(out=po, lhsT=vt[0][:, 0:SPC],
                                         rhs=r3[:, 0, hs],
                                         start=True, stop=False)
                        nc.tensor.matmul(out=po, lhsT=vt[1][:, 0:SPC],
                                         rhs=r3[:, 1, hs],
                                         start=False, stop=False)
                        nc.tensor.matmul(out=po, lhsT=vt2[:, 0:SPC],
                                         rhs=r3[0:65, 2, hs],
                                         start=False, stop=True)
                    nc.vector.tensor_scalar(
                        out=ot[:], in0=pot[:, 0:500],
                        scalar1=sinv[:, 0:1], scalar2=None, op0=ALU.mult)
                    for h in (0, 1):
                        nc.sync.dma_start(
                            out_d.ap()[:, j * OW + h * 500 :
                                       j * OW + h * 500 + 500],
                            ot[64 * h : 64 * h + SPC, :])
                else:
                    ot = otp.tile([SPC, OW], dt.float32, tag="ot", name="ot")
                    for s5 in range(OW // 500):
                        c0 = s5 * 500
                        pot = pfp.tile([128, 512], dt.float32, tag="psf",
                                       name="psf")
                        po = pot[0:SPC, 0:500]
                        nc.tensor.matmul(out=po, lhsT=vt[0][:, 0:SPC],
                                         rhs=r3[:, 0, c0 : c0 + 500],
                                         start=True, stop=False)
                        nc.tensor.matmul(out=po, lhsT=vt[1][:, 0:SPC],
                                         rhs=r3[:, 1, c0 : c0 + 500],
                                         start=False, stop=False)
                        nc.tensor.matmul(out=po, lhsT=vt2[:, 0:SPC],
                                         rhs=r3[0:65, 2, c0 : c0 + 500],
                                         start=False, stop=True)
                        nc.vector.tensor_scalar(
                            out=ot[:, c0 : c0 + 500], in0=po,
                            scalar1=sinv[0:SPC, 0:1], scalar2=None,
                            op0=ALU.mult)
                    nc.sync.dma_start(out_d.ap()[:, j * OW : (j + 1) * OW],
                                      ot[:])

    nc.compile()
    return nc


# ---- top-level entry ------------------------------------------------------

_CACHE = {}


def _build_in_maps(meta, cores, sh):
    in_maps = []
    for d in cores:
        m = {
            "sub_lo": sh["sub_lo"], "sub_hi": sh["sub_hi"],
            "node_t": sh["node_t"],
            "ramp": sh["ramp"], "a_bc": sh["a_bc"],
            "fcw": sh["fcw"], "outw": sh["outw"], "iddup": sh["iddup"],
            "xidx": d["xidx"], "v0idx": d["v0idx"], "v8idx": d["v8idx"],
            "sampid": d["sampid"],
        }
        for dd in ("f", "b"):
            m[f"tbl_{dd}"] = sh[f"tbl_{dd}"]
            m[f"tbl_{dd}h"] = sh[f"tbl_{dd}h"]
            m[f"tbl_{dd}c"] = sh[f"tbl_{dd}c"]
            m[f"wih_{dd}"] = sh[f"wih_{dd}"]
            m[f"whh_{dd}"] = sh[f"whh_{dd}"]
            m[f"bias_{dd}"] = sh[f"bias_{dd}"]
        for side in ("ll", "rl"):
            for cls in ("lo", "hi"):
                m[f"{side}_{cls}_idx"] = d[f"{side}_{cls}_idx"]
                m[f"{side}_{cls}_ctx"] = d[f"{side}_{cls}_ctx"]
        in_maps.append(m)
    return in_maps


def _shapes_of(in_map):
    import concourse.mybir as mybir
    dt = mybir.dt
    np2my = {
        np.dtype(np.float32): dt.float32,
        np.dtype(BF16): dt.bfloat16,
        np.dtype(np.int16): dt.int16,
    }
    return [(k, list(v.shape), np2my[v.dtype]) for k, v in sorted(in_map.items())]


def kernel(**inputs):
    from concourse.bass_utils import run_bass_kernel_spmd

    inp = {k: np.asarray(v) for k, v in inputs.items()}
    meta, cores = prep_all(inp)
    sh = prep_shared(inp)
    in_maps = _build_in_maps(meta, cores, sh)
    shapes = _shapes_of(in_maps[0])
    key = (str(shapes) + str(meta["np"]) + f"{GX}{HCMERGE}{ATT_PSUM}{TP64}{XSTREAM}{TTR}{ILV}"
           + str({k: v.tolist() for k, v in meta["nb"].items()}))
    if key not in _CACHE:
        _CACHE[key] = build_nc(meta, shapes)
    nc = _CACHE[key]
    res = run_bass_kernel_spmd(nc, in_maps, core_ids=list(range(N_CORES)))
    return np.concatenate([res.results[c]["out"] for c in range(N_CORES)], axis=0)



# revision 56
# speedup vs baseline: 1.0507x; 1.0405x over previous
"""Trainium2 Bass kernel for nn_C2SModel (code2seq-style model), v2.

Optimizations over baseline:
  - t=0 LSTM step collapsed to a 512-entry vocab table (h1,c1), built on
    device, DMA-gathered per span.
  - Cell elementwise via scalar_tensor_tensor (4x DVE mode).
  - Phase C (fc + attention) pipelined into the span loop; exp computed
    via tanh identity to keep one activation table set.
  - Dynamic NP (pad to actual max contexts per core).
  - Batched gathers (1024-row calls).
"""
import os
import numpy as np
import ml_dtypes
from contextlib import ExitStack

BF16 = ml_dtypes.bfloat16
T0_TABLE = os.environ.get("NO_T0_TABLE", "") != "1"
GX = int(os.environ.get("GX", "512"))  # gather rows per call (1024 crashes HW)
HCMERGE = os.environ.get("USE_HCMERGE", "") == "1"  # needs 1024-row gather: crashes
ATT_PSUM = os.environ.get("NO_ATT_PSUM", "") != "1"
TP64 = os.environ.get("NO_TP64", "") != "1"     # 128-part out chunks
XSTREAM = os.environ.get("NO_XSTREAM", "") != "1"
TTR = os.environ.get("USE_TTR", "") == "1"  # InstTensorTensorReduce crashes on HW
ILV = os.environ.get("NO_ILV", "") != "1"       # interleaved emission

# ---- problem constants (hardcoded per contract) ---------------------------
N_CTX = 65536
B = 512
E = 128
H = 128
T = 9
SUB_V = 50000
NODE_V = 512
DEC = 320
OUT_D = 10000
N_CORES = 8
SPC = B // N_CORES            # 64 samples per core
SPLIT = 32768                 # subtoken table split (int16 gather indices)
GROUP = 4                     # ctx-tiles of 128 per gather call group
OUT_NCH = 500                 # final matmul N-chunk (20 chunks of 500)


# ---- host-side prep -------------------------------------------------------

def _wrap_idx(ids):
    ids = np.asarray(ids, np.int16)
    assert len(ids) % 16 == 0
    w16 = ids.reshape(-1, 16).T
    return np.tile(w16, (8, 1)).copy()


def _pad_to(x, n, val):
    out = np.full((n,) + x.shape[1:], val, x.dtype)
    out[: len(x)] = x
    return out


def _core_raw(k, inp, NP):
    """Per-core raw occurrence lists and context data."""
    NT128 = NP // 128
    indices = inp["indices"]
    s = int(np.searchsorted(indices, k * SPC, "left"))
    e = int(np.searchsorted(indices, (k + 1) * SPC, "left"))
    nk = e - s
    assert nk <= NP, f"core {k}: {nk} contexts exceed NP={NP}"
    samp = _pad_to(indices[s:e].astype(np.int32) - k * SPC, NP, -1)
    pth = _pad_to(inp["paths"][s:e].astype(np.int16), NP, 0)
    d = {"samp": samp, "paths": pth, "nk": nk}
    for side in ("ll", "rl"):
        idxs_g = inp[f"{side}_indices"]
        subs_g = inp[f"{side}_subtokens"]
        o_s = int(np.searchsorted(idxs_g, s, "left"))
        o_e = int(np.searchsorted(idxs_g, e, "left"))
        subs = subs_g[o_s:o_e].astype(np.int32)
        ctxs = idxs_g[o_s:o_e].astype(np.int32) - s
        tb = np.searchsorted(ctxs, np.arange(0, NP + 128, 128))
        per_tile = {"lo": [], "hi": []}
        for t in range(NT128):
            sl = slice(tb[t], tb[t + 1])
            tsub, tctx = subs[sl], ctxs[sl] - t * 128
            m = tsub < SPLIT
            per_tile["lo"].append((tsub[m], tctx[m]))
            per_tile["hi"].append((tsub[~m] - SPLIT, tctx[~m]))
        d[side] = per_tile
    return d


def prep_all(inp):
    """Returns (meta, per_core_data)."""
    indices = inp["indices"]
    counts = [
        int(np.searchsorted(indices, (k + 1) * SPC, "left"))
        - int(np.searchsorted(indices, k * SPC, "left"))
        for k in range(N_CORES)
    ]
    NP = ((max(counts) + 127) // 128) * 128   # pad to 128 multiple
    NT128 = NP // 128
    raws = [_core_raw(k, inp, NP) for k in range(N_CORES)]
    nb = {}
    for side in ("ll", "rl"):
        for cls in ("lo", "hi"):
            nb[(side, cls)] = np.array(
                [
                    max((len(r[side][cls][t][0]) + 127) // 128 for r in raws)
                    for t in range(NT128)
                ],
                np.int32,
            )
    meta = {"nb": nb, "np": NP}
    SPANS = [(n, min(1024, NP - n)) for n in range(0, NP, 1024)]

    cores = []
    for r in raws:
        d = {"samp": r["samp"], "nk": r["nk"]}
        pth = r["paths"]
        xidx = np.concatenate(
            [pth[n0 : n0 + W].T.reshape(-1) for (n0, W) in SPANS]
        )
        d["xidx"] = _wrap_idx(xidx)
        d["v0idx"] = _wrap_idx(pth[:, 0].copy())
        d["v8idx"] = _wrap_idx(pth[:, T - 1].copy())
        d["sampid"] = np.ascontiguousarray(
            r["samp"].reshape(NT128, 128).T.astype(np.float32)
        )
        for side in ("ll", "rl"):
            for cls in ("lo", "hi"):
                subs_s, ctxs_s = [], []
                for t in range(NT128):
                    ts_, tc_ = r[side][cls][t]
                    n = nb[(side, cls)][t] * 128
                    subs_s.append(_pad_to(ts_.astype(np.int32), n, 0))
                    ctxs_s.append(_pad_to(tc_.astype(np.int32), n, -1))
                subs_s = np.concatenate(subs_s) if subs_s else np.zeros(0, np.int32)
                ctxs_s = np.concatenate(ctxs_s) if ctxs_s else np.zeros(0, np.int32)
                d[f"{side}_{cls}_idx"] = _wrap_idx(subs_s.astype(np.int16))
                d[f"{side}_{cls}_ctx"] = np.ascontiguousarray(
                    ctxs_s.reshape(-1, 128).T.astype(np.float32)
                )
        cores.append(d)
    return meta, cores


def prep_shared(inp):
    """Replicated (same for all cores) tensors."""
    sub = np.asarray(inp["subtoken_emb"], np.float32)
    node = np.asarray(inp["node_emb"], np.float32)
    sh = {
        "sub_lo": sub[:SPLIT].astype(BF16),
        "sub_hi": sub[SPLIT:].astype(BF16),
        "node_t": node.astype(BF16),
        "nodeT": np.ascontiguousarray(node.T).astype(BF16),  # [128 e, 512 v]
        "ramp": np.tile(np.arange(128, dtype=np.float32), (128, 1)).astype(BF16),
        "iotap": np.tile(
            np.arange(128, dtype=np.float32)[:, None], (1, 8)
        ),  # [128, 8] f32, col j = partition idx
        "a_bc": np.tile(np.asarray(inp["a"], np.float32), (128, 1)).astype(BF16),
        # [64,128] fp32: iddup[k,m] = (k == m%64); broadcasts the per-sample
        # sum-of-exp column to 128 partitions via one tiny fp32 matmul
        "iddup": (np.arange(64)[:, None] == (np.arange(128)[None, :] % 64))
        .astype(np.float32),
    }
    def _sig(x):
        return 1.0 / (1.0 + np.exp(-x))

    for d in ("f", "b"):
        # t=0 LSTM step for all 512 path-node vocab entries (h=c=0):
        # c1 = sig(i)*tanh(g); h1 = sig(o)*tanh(c1)   (gate order i,f,g,o)
        wih = np.asarray(inp[f"w_ih_{d}"], np.float32)
        bias0 = (np.asarray(inp[f"b_ih_{d}"], np.float32)
                 + np.asarray(inp[f"b_hh_{d}"], np.float32))
        g0 = node @ wih.T + bias0                        # [512, 512]
        c1 = _sig(g0[:, 0:128]) * np.tanh(g0[:, 256:384])
        h1 = _sig(g0[:, 384:512]) * np.tanh(c1)
        # combined h|c table: one 256-elem gather fetches both states
        sh[f"tbl_{d}"] = np.ascontiguousarray(
            np.concatenate([h1, c1], axis=1)
        ).astype(BF16)
        sh[f"tbl_{d}h"] = np.ascontiguousarray(h1).astype(BF16)
        sh[f"tbl_{d}c"] = np.ascontiguousarray(c1).astype(BF16)
        sh[f"wih_{d}"] = np.ascontiguousarray(
            np.asarray(inp[f"w_ih_{d}"], np.float32).T
        ).astype(BF16)
        sh[f"whh_{d}"] = np.ascontiguousarray(
            np.asarray(inp[f"w_hh_{d}"], np.float32).T
        ).astype(BF16)
        bias = np.asarray(inp[f"b_ih_{d}"], np.float32) + np.asarray(
            inp[f"b_hh_{d}"], np.float32
        )
        sh[f"bias_{d}"] = np.ascontiguousarray(bias.reshape(4, 128).T)  # [128,4]
    fcwT = np.asarray(inp["fc_w"], np.float32).T
    sh["fcw"] = np.ascontiguousarray(
        fcwT.reshape(4, 128, DEC).transpose(1, 0, 2).reshape(128, 4 * DEC)
    ).astype(BF16)
    outw = np.concatenate(
        [np.asarray(inp["out_w"], np.float32).T,
         np.asarray(inp["out_b"], np.float32)[None, :]], axis=0
    )  # [321, 10000]
    outwP = np.zeros((128, 3 * OUT_D), np.float32)
    for c in range(3):
        rows = outw[c * 128 : min((c + 1) * 128, 321)]
        outwP[: len(rows), c * OUT_D : (c + 1) * OUT_D] = rows
    sh["outw"] = outwP.astype(BF16)
    return sh


# ---- bass program ---------------------------------------------------------

def build_nc(meta, shapes):
    import concourse.bass as bass
    import concourse.bacc as bacc
    import concourse.tile as tile
    import concourse.mybir as mybir
    from concourse.library_config import mlp as mlp_lib

    dt = mybir.dt
    AF = mybir.ActivationFunctionType
    ALU = mybir.AluOpType
    nb = meta["nb"]
    NP = meta["np"]
    NT128 = NP // 128
    SPANS = [(n, min(1024, NP - n)) for n in range(0, NP, 1024)]

    nc = bacc.Bacc("TRN2", target_bir_lowering=False, debug=False,
                   num_devices=N_CORES,
                   dynamic_dma_scratch_size=32768)

    dr = {}
    for name, arr_shape, dtype in shapes:
        dr[name] = nc.dram_tensor(name, list(arr_shape), dtype,
                                  kind="ExternalInput")
    out_d = nc.dram_tensor("out", [SPC, OUT_D], dt.float32, kind="ExternalOutput")

    boff = {}
    for key, arr in nb.items():
        boff[key] = np.concatenate([[0], np.cumsum(arr)])

    with tile.TileContext(nc) as tc, ExitStack() as ctx:
        nc.gpsimd.load_library(mlp_lib)

        cp = ctx.enter_context(tc.tile_pool(name="const", bufs=1))

        def load_const(name):
            h = dr[name]
            t = cp.tile(list(h.shape), h.dtype, tag=name)
            nc.sync.dma_start(t[:], h.ap()[:, :])
            return t

        # load order = DMA queue order: unblock gathers + table build first
        w = {}
        for d in ("f", "b"):
            w[f"wih_{d}"] = load_const(f"wih_{d}")
            w[f"whh_{d}"] = load_const(f"whh_{d}")
            w[f"bias_{d}"] = load_const(f"bias_{d}")
        ramp = load_const("ramp")
        v0idx = load_const("v0idx")
        v8idx = load_const("v8idx")
        ctxid = {}
        sidx = {}
        a_bc = fcw = sampid = iddup = None
        xidx_full = [None]
        if not XSTREAM:
            xidx_full[0] = load_const("xidx")

        def load_late():
            # phase A/C constants aren't needed until mid-span-0; loading
            # them after span-0's gather stream keeps HWDGE off the LSTM's
            # critical path at startup
            nonlocal a_bc, fcw, sampid, iddup
            for side in ("ll", "rl"):
                for cls in ("lo", "hi"):
                    ctxid[(side, cls)] = load_const(f"{side}_{cls}_ctx")
                    sidx[(side, cls)] = load_const(f"{side}_{cls}_idx")
            a_bc = load_const("a_bc")
            fcw = load_const("fcw")
            sampid = load_const("sampid")
            iddup = load_const("iddup")

        big = ctx.enter_context(tc.tile_pool(name="big", bufs=1))
        embT = {s: big.tile([128, NP], dt.bfloat16, tag=f"embT_{s}",
                            name=f"embT_{s}") for s in ("ll", "rl")}
        hT = {d: big.tile([128, NP], dt.bfloat16, tag=f"hT_{d}",
                          name=f"hT_{d}") for d in ("f", "b")}

        # ---- pools -----------------------------------------------------
        # outer pools first, then span-loop pools in an inner scope that is
        # freed before the tail (stack order respected)
        csp = ctx.enter_context(tc.tile_pool(name="cstage", bufs=2))
        wp = ctx.enter_context(tc.tile_pool(name="wstream", bufs=3))
        pfp = ctx.enter_context(tc.tile_pool(name="psF", bufs=2, space="PSUM"))
        atp = ctx.enter_context(tc.tile_pool(name="psAtt", bufs=1, space="PSUM"))
        sctx = ctx.enter_context(ExitStack())
        gp = sctx.enter_context(tc.tile_pool(name="gdst", bufs=2))
        ohp = sctx.enter_context(tc.tile_pool(name="oh", bufs=2))
        xp = sctx.enter_context(tc.tile_pool(name="xt", bufs=2))
        hgp = sctx.enter_context(tc.tile_pool(name="hcg", bufs=2))
        sp = sctx.enter_context(tc.tile_pool(name="stage", bufs=2))
        hcp = sctx.enter_context(tc.tile_pool(name="hc", bufs=2))
        pap = sctx.enter_context(tc.tile_pool(name="psA", bufs=1, space="PSUM"))
        pbp = sctx.enter_context(tc.tile_pool(name="psB", bufs=2, space="PSUM"))

        # attention accumulator in a dedicated PSUM bank, accumulated
        # directly by the per-tile matmuls (start=False onto a memset bank):
        # cols 0:65 = v-chunk0, 65:130 = v-chunk1, 130:195 = v-chunk2
        # (rows<65), 195:196 = sum-of-exp (rows<65)
        if ATT_PSUM:
            att = atp.tile([128, 512], dt.float32, tag="att", name="att")
            nc.vector.memset(att[:, 0:256], 0)
        else:
            att = big.tile([128, 212], dt.float32, tag="att", name="att")
            nc.vector.memset(att[:], 0)

        # ================= phase A: subtoken segment sums ================
        def emit_A_gathers(grps):
            dsts = {}
            for side in ("ll", "rl"):
                tblmap = {"lo": dr["sub_lo"], "hi": dr["sub_hi"]}
                for g in grps:
                    t0, t1 = g * GROUP, min((g + 1) * GROUP, NT128)
                    dst = {}
                    for cls in ("lo", "hi"):
                        b0, b1 = boff[(side, cls)][t0], boff[(side, cls)][t1]
                        nblk = int(b1 - b0)
                        if nblk == 0:
                            continue
                        chunks = []
                        for c0 in range(0, nblk, 8):
                            cn = min(8, nblk - c0)
                            dtile = gp.tile([128, 8 * 128], dt.bfloat16,
                                            tag=f"g_{cls}", name=f"g_{cls}",
                                            bufs=4)
                            d3 = dtile[:].rearrange("p (b e) -> p b e", e=128)
                            nc.gpsimd.dma_gather(
                                d3[:, 0:cn, :],
                                tblmap[cls].ap()[:, :],
                                sidx[(side, cls)][:, (int(b0) + c0) * 8 :
                                                  (int(b0) + c0 + cn) * 8],
                                cn * 128,
                                cn * 128,
                                128,
                            )
                            chunks.append(dtile)
                        dst[cls] = (chunks, int(b0))
                    dsts[(side, g)] = dst
            return dsts

        def emit_A_compute_one(side, g, dsts):
                    t0, t1 = g * GROUP, min((g + 1) * GROUP, NT128)
                    dst = dsts[(side, g)]
                    for t in range(t0, t1):
                        blocks = []
                        for cls in ("lo", "hi"):
                            nbt = int(nb[(side, cls)][t])
                            if nbt == 0:
                                continue
                            chunks, gb0 = dst[cls]
                            tb0 = int(boff[(side, cls)][t])
                            for j in range(nbt):
                                lb = tb0 - gb0 + j
                                # (chunk tile, block within chunk, ctxid col)
                                blocks.append((chunks[lb // 8], lb % 8,
                                               ctxid[(side, cls)], tb0 + j))
                        if not blocks:
                            nc.vector.memset(embT[side][:, t * 128 : (t + 1) * 128], 0)
                            continue
                        ps = pap.tile([128, 128], dt.float32, tag="psA",
                                      name="psA")
                        for j, (dtile, lb, cid, gcol) in enumerate(blocks):
                            oh = ohp.tile([128, 128], dt.bfloat16, tag="oh",
                                          name="oh")
                            nc.vector.tensor_scalar(
                                out=oh[:], in0=ramp[:],
                                scalar1=cid[:, gcol : gcol + 1],
                                scalar2=None, op0=ALU.is_equal,
                            )
                            nc.tensor.matmul(
                                out=ps[:],
                                lhsT=dtile[:, lb * 128 : (lb + 1) * 128],
                                rhs=oh[:],
                                start=(j == 0),
                                stop=(j == len(blocks) - 1),
                            )
                        nc.vector.tensor_copy(
                            out=embT[side][:, t * 128 : (t + 1) * 128], in_=ps[:]
                        )

        def emit_A_tasks(grps, dsts):
            return [
                (lambda side=side, g=g: emit_A_compute_one(side, g, dsts))
                for side in ("ll", "rl")
                for g in grps
            ]

        # ================= phase B: bidirectional LSTM ===================
        def emit_gathers(n0, W):
            """x + h1/c1 gathers for a span; returns (xt, hp, cp_)."""
            # stream this span's gather indices (a small DMA) instead of
            # front-loading the whole 1.2MB xidx table before span 0
            if XSTREAM:
                xi = wp.tile([128, (1024 * T) // 16], dt.int16, tag="xidx",
                             name="xis", bufs=2)
                nc.sync.dma_start(
                    xi[:, 0 : (W * T) // 16],
                    dr["xidx"].ap()[:, (n0 * T) // 16 : ((n0 + W) * T) // 16])
                xoff = 0
            else:
                xi = xidx_full[0]
                xoff = (n0 * T) // 16
            xt = xp.tile([128, T * 1024], dt.bfloat16, tag="xt", name="xt")
            x3 = xt[:].rearrange("p (a n) -> p a n", a=1)
            hp, cp_ = {}, {}
            if T0_TABLE:
                for d, vids in (("f", v0idx), ("b", v8idx)):
                    if HCMERGE:
                        # combined [512,256] h|c table: one gather per dir
                        t_ = hgp.tile([128, 2 * W], dt.bfloat16,
                                      tag=f"hc_{d}", name=f"hc_{d}")
                        t3 = t_[:].rearrange("p (a n) -> p a n", a=2)
                        nc.gpsimd.dma_gather(
                            t3[:, :, :],
                            dr[f"tbl_{d}"].ap()[:, :],
                            vids[:, n0 // 16 : (n0 + W) // 16],
                            W, W, 2 * H, transpose=True,
                        )
                        hp[d] = t_[:, 0:W]
                        cp_[d] = t_[:, W : 2 * W]
                        continue
                    for hc_, store in (("h", hp), ("c", cp_)):
                        t_ = hgp.tile([128, 1024], dt.bfloat16,
                                      tag=f"hc_{d}{hc_}", name=f"hc_{d}{hc_}")
                        t3 = t_[:].rearrange("p (a n) -> p a n", a=1)
                        for h2 in range(0, W, GX):
                            cw = min(GX, W - h2)
                            nc.gpsimd.dma_gather(
                                t3[:, :, h2 : h2 + cw],
                                dr[f"tbl_{d}{hc_}"].ap()[:, :],
                                vids[:, (n0 + h2) // 16 : (n0 + h2 + cw) // 16],
                                cw, cw, H, transpose=True,
                            )
                        store[d] = t_[:, 0:W]
            # x gathered in LSTM consumption order (t=1 needs tt=1 and 7, ...)
            for tg_ in (1, 7, 2, 6, 3, 5, 4, 8, 0):
                for h2 in range(0, W, GX):
                    cw = min(GX, W - h2)
                    nc.gpsimd.dma_gather(
                        x3[:, :, tg_ * W + h2 : tg_ * W + h2 + cw],
                        dr["node_t"].ap()[:, :],
                        xi[:, xoff + (tg_ * W + h2) // 16 :
                           xoff + (tg_ * W + h2 + cw) // 16],
                        cw, cw, 128, transpose=True,
                    )
            return xt, hp, cp_

        def emit_B_tasks(n0, W, gathered):
            xt, hp, cp_ = gathered
            tasks = []

            def t0_fallback():
                # debug fallback: plain t=0 step (no f-gate, x-only matmuls)
                for d in ("f", "b"):
                    tt = 0 if d == "f" else (T - 1)
                    bias = w[f"bias_{d}"]
                    pgs = {}
                    for gi in (0, 2, 3):
                        pg = pbp.tile([128, 1024], dt.float32, tag="psB",
                                      name="psB")
                        for h2 in range(0, W, 512):
                            cw = min(512, W - h2)
                            nc.tensor.matmul(
                                out=pg[:, h2 : h2 + cw],
                                lhsT=w[f"wih_{d}"][:, gi * 128 : (gi + 1) * 128],
                                rhs=xt[:, tt * W + h2 : tt * W + h2 + cw],
                                start=True, stop=True,
                            )
                        pgs[gi] = pg
                    si = sp.tile([128, 1024], dt.bfloat16, tag="si", name="si")
                    nc.scalar.activation(si[:, 0:W], pgs[0][:, 0:W], AF.Sigmoid,
                                         bias=bias[:, 0:1])
                    tg = sp.tile([128, 1024], dt.bfloat16, tag="tg", name="tg")
                    nc.scalar.activation(tg[:, 0:W], pgs[2][:, 0:W], AF.Tanh,
                                         bias=bias[:, 2:3])
                    so = sp.tile([128, 1024], dt.bfloat16, tag="so", name="so")
                    nc.scalar.activation(so[:, 0:W], pgs[3][:, 0:W], AF.Sigmoid,
                                         bias=bias[:, 3:4])
                    c_new = hcp.tile([128, 1024], dt.bfloat16, tag="c", name="c")
                    nc.vector.tensor_tensor(out=c_new[:, 0:W], in0=si[:, 0:W],
                                            in1=tg[:, 0:W], op=ALU.mult)
                    nc.scalar.activation(tg[:, 0:W], c_new[:, 0:W], AF.Tanh)
                    h_new = hcp.tile([128, 1024], dt.bfloat16, tag="h", name="h")
                    nc.vector.tensor_tensor(out=h_new[:, 0:W], in0=so[:, 0:W],
                                            in1=tg[:, 0:W], op=ALU.mult)
                    hp[d], cp_[d] = h_new[:, 0:W], c_new[:, 0:W]

            if not T0_TABLE:
                tasks.append(t0_fallback)

            def step(t):
                for d in ("f", "b"):
                    h_prev = hp[d]
                    c_prev = cp_[d]
                    tt = t if d == "f" else (T - 1 - t)
                    bias = w[f"bias_{d}"]
                    psg = []
                    for gi in range(4):
                        pg = pbp.tile([128, 1024], dt.float32, tag="psB",
                                      name="psB")
                        for h2 in range(0, W, 512):
                            cw = min(512, W - h2)
                            nc.tensor.matmul(
                                out=pg[:, h2 : h2 + cw],
                                lhsT=w[f"wih_{d}"][:, gi * 128 : (gi + 1) * 128],
                                rhs=xt[:, tt * W + h2 : tt * W + h2 + cw],
                                start=True, stop=False,
                            )
                            nc.tensor.matmul(
                                out=pg[:, h2 : h2 + cw],
                                lhsT=w[f"whh_{d}"][:, gi * 128 : (gi + 1) * 128],
                                rhs=h_prev[:, h2 : h2 + cw],
                                start=False, stop=True,
                            )
                        psg.append(pg)
                    si = sp.tile([128, 1024], dt.bfloat16, tag="si", name="si")
                    nc.scalar.activation(si[:, 0:W], psg[0][:, 0:W], AF.Sigmoid,
                                         bias=bias[:, 0:1])
                    sf = sp.tile([128, 1024], dt.bfloat16, tag="sf", name="sf")
                    nc.scalar.activation(sf[:, 0:W], psg[1][:, 0:W], AF.Sigmoid,
                                         bias=bias[:, 1:2])
                    tg = sp.tile([128, 1024], dt.bfloat16, tag="tg", name="tg")
                    nc.scalar.activation(tg[:, 0:W], psg[2][:, 0:W], AF.Tanh,
                                         bias=bias[:, 2:3])
                    so = sp.tile([128, 1024], dt.bfloat16, tag="so", name="so")
                    nc.scalar.activation(so[:, 0:W], psg[3][:, 0:W], AF.Sigmoid,
                                         bias=bias[:, 3:4])
                    # in-place: u = si*tg -> si ; v = sf*c -> sf ; th -> tg
                    nc.vector.tensor_tensor(out=si[:, 0:W], in0=si[:, 0:W],
                                            in1=tg[:, 0:W], op=ALU.mult)
                    nc.vector.tensor_tensor(out=sf[:, 0:W], in0=sf[:, 0:W],
                                            in1=c_prev[:, 0:W], op=ALU.mult)
                    c_new = hcp.tile([128, 1024], dt.bfloat16, tag="c", name="c")
                    nc.vector.tensor_tensor(out=c_new[:, 0:W], in0=si[:, 0:W],
                                            in1=sf[:, 0:W], op=ALU.add)
                    nc.scalar.activation(tg[:, 0:W], c_new[:, 0:W], AF.Tanh)
                    if t == T - 1:
                        h_new = None
                        nc.vector.tensor_tensor(
                            out=hT[d][:, n0 : n0 + W], in0=so[:, 0:W],
                            in1=tg[:, 0:W], op=ALU.mult)
                    else:
                        h_new = hcp.tile([128, 1024], dt.bfloat16, tag="h",
                                         name="h")
                        nc.vector.tensor_tensor(
                            out=h_new[:, 0:W], in0=so[:, 0:W],
                            in1=tg[:, 0:W], op=ALU.mult)
                    hp[d], cp_[d] = (h_new[:, 0:W] if h_new is not None else None,
                                     c_new[:, 0:W])

            for t in range(1, T):
                tasks.append(lambda t=t: step(t))
            return tasks

        # ====== phase C: fc + attention (per span, accumulates att) ======
        STOPF = {"v": False}

        def emit_C(tiles128):
            feats = [embT["ll"], hT["f"], hT["b"], embT["rl"]]
            nts = len(tiles128)
            cxts = []
            score = csp.tile([128, 16], dt.float32, tag="score", name="score")
            for ti, t in enumerate(tiles128):
                sl = slice(t * 128, (t + 1) * 128)
                psf = pfp.tile([128, 512], dt.float32, tag="psf", name="psf")
                for gi in range(4):
                    nc.tensor.matmul(
                        out=psf[:, 0:DEC],
                        lhsT=feats[gi][:, sl],
                        rhs=fcw[:, gi * DEC : (gi + 1) * DEC],
                        start=(gi == 0), stop=(gi == 3),
                    )
                cxt = csp.tile([128, 322], dt.bfloat16, tag="cxt", name="cxt",
                               bufs=4)
                nc.scalar.activation(cxt[:, 0:DEC], psf[:, 0:DEC], AF.Tanh)
                nc.vector.memset(cxt[:, DEC : DEC + 2], 1.0)
                prod = csp.tile([128, 322], dt.bfloat16, tag="ectx", name="prod")
                if TTR:
                    nc.vector.tensor_tensor_reduce(
                        out=prod[:, 0:DEC], in0=cxt[:, 0:DEC], in1=a_bc[:],
                        scale=1.0, scalar=0.0, op0=ALU.mult, op1=ALU.add,
                        accum_out=score[:, ti : ti + 1],
                    )
                else:
                    nc.vector.tensor_tensor(out=prod[:, 0:DEC],
                                            in0=cxt[:, 0:DEC],
                                            in1=a_bc[:], op=ALU.mult)
                    nc.vector.tensor_reduce(out=score[:, ti : ti + 1],
                                            in_=prod[:, 0:DEC],
                                            axis=mybir.AxisListType.X,
                                            op=ALU.add)
                cxts.append(cxt)
            # exp(score) via tanh identity, batched over the span's tiles:
            # th = tanh(score/2); e = (1+th)/(1-th)
            sth = score
            nc.scalar.activation(sth[:, 0:nts], score[:, 0:nts], AF.Tanh,
                                 scale=0.5)
            den = csp.tile([128, 16], dt.float32, tag="den", name="den")
            nc.vector.tensor_scalar(out=den[:, 0:nts], in0=sth[:, 0:nts],
                                    scalar1=-1.0, scalar2=1.0,
                                    op0=ALU.mult, op1=ALU.add)
            rden = csp.tile([128, 16], dt.float32, tag="rden", name="rden")
            nc.vector.reciprocal(rden[:, 0:nts], den[:, 0:nts])
            ee = csp.tile([128, 16], dt.float32, tag="ee", name="ee")
            nc.vector.scalar_tensor_tensor(
                out=ee[:, 0:nts], in0=sth[:, 0:nts], scalar=1.0,
                in1=rden[:, 0:nts], op0=ALU.add, op1=ALU.mult)
            for ti, t in enumerate(tiles128):
                cxt = cxts[ti]
                ectx = csp.tile([128, 322], dt.bfloat16, tag="ectx", name="ectx")
                nc.vector.tensor_scalar(
                    out=ectx[:, 0:322], in0=cxt[:, 0:322],
                    scalar1=ee[:, ti : ti + 1], scalar2=None, op0=ALU.mult,
                )
                ohb = csp.tile([128, 65], dt.bfloat16, tag="ohb", name="ohb")
                nc.vector.tensor_scalar(
                    out=ohb[:], in0=ramp[:, 0:65],
                    scalar1=sampid[:, t : t + 1], scalar2=None,
                    op0=ALU.is_equal,
                )
                if ATT_PSUM:
                    stp = STOPF["v"] and (ti == nts - 1)
                    nc.tensor.matmul(out=att[:, 0:65], lhsT=ectx[:, 0:128],
                                     rhs=ohb[:], start=False, stop=stp,
                                     skip_group_check=True)
                    nc.tensor.matmul(out=att[:, 65:130], lhsT=ectx[:, 128:256],
                                     rhs=ohb[:], start=False, stop=stp,
                                     skip_group_check=True)
                    nc.tensor.matmul(out=att[0:65, 130:195],
                                     lhsT=ectx[:, 256:321],
                                     rhs=ohb[:], start=False, stop=stp,
                                     skip_group_check=True)
                    nc.tensor.matmul(out=att[0:65, 195:196], lhsT=ohb[:],
                                     rhs=ectx[:, 320:321], start=False,
                                     stop=stp, skip_group_check=True)
                else:
                    patt = pfp.tile([128, 512], dt.float32, tag="psf",
                                    name="psf")
                    nc.tensor.matmul(out=patt[:, 0:65], lhsT=ectx[:, 0:128],
                                     rhs=ohb[:], start=True, stop=True)
                    nc.tensor.matmul(out=patt[:, 65:130], lhsT=ectx[:, 128:256],
                                     rhs=ohb[:], start=True, stop=True)
                    nc.tensor.matmul(out=patt[0:65, 130:195],
                                     lhsT=ectx[:, 256:321],
                                     rhs=ohb[:], start=True, stop=True)
                    nc.tensor.matmul(out=patt[0:65, 195:196], lhsT=ohb[:],
                                     rhs=ectx[:, 320:321], start=True,
                                     stop=True)
                    nc.vector.tensor_tensor(out=att[:, 0:130],
                                            in0=att[:, 0:130],
                                            in1=patt[:, 0:130], op=ALU.add)
                    nc.vector.tensor_tensor(out=att[0:65, 130:196],
                                            in0=att[0:65, 130:196],
                                            in1=patt[0:65, 130:196],
                                            op=ALU.add)

        # ================= main loop =====================================
        # Interleave: LSTM steps of span s carry (a) phase-C groups of span
        # s-1 and (b) phase-A compute of span s between them, so the ACT and
        # DVE queues never drain while PE refills psums.
        def emit_span(n0, W, g, dsts, extra):
            grps = range(n0 // 512, (n0 + W + 511) // 512)
            btasks = emit_B_tasks(n0, W, g)
            # round-robin A (this span) with C (previous span): A frees the
            # gather-dest tiles the NEXT span's Pool queue is waiting on,
            # C keeps the ACT queue full
            atasks = emit_A_tasks(grps, dsts)
            work = []
            for i in range(max(len(atasks), len(extra))):
                if i < len(atasks):
                    work.append(atasks[i])
                if i < len(extra):
                    work.append(extra[i])
            nt = (n0 + W) // 128
            ctasks = [
                (lambda c0=c0: emit_C(range(c0, min(c0 + 3, nt))))
                for c0 in range(n0 // 128, nt, 3)
            ]
            if not ILV:
                for bt in btasks:
                    bt()
                for w_ in work:
                    w_()
                return ctasks
            nb_ = len(btasks)
            for i, bt in enumerate(btasks):
                bt()
                lo = (i * len(work)) // nb_
                hi = ((i + 1) * len(work)) // nb_
                for w_ in work[lo:hi]:
                    w_()
            return ctasks

        # issue span s+1's gathers ahead of span s's compute so the Pool
        # queue keeps the LSTM fed across span boundaries
        pend = None
        pendC = []
        OW = 1000
        for si_, (n0, W) in enumerate(SPANS):
            g = emit_gathers(n0, W)
            if si_ == 0:
                load_late()
            dsts = emit_A_gathers(range(n0 // 512, (n0 + W + 511) // 512))
            if pend is not None:
                pendC = emit_span(*pend, pendC)
            pend = (n0, W, g, dsts)
        pendC = emit_span(*pend, pendC)
        for i, w_ in enumerate(pendC):
            if i == len(pendC) - 1:
                STOPF["v"] = True
            w_()

        # ================= finalize: v, S, output matmul =================
        sctx.close()   # free span-loop pools for the outw stream
        vt = [csp.tile([128, 65], dt.bfloat16, tag=f"vt{c}", name=f"vt{c}")
              for c in range(2)]
        vt2 = csp.tile([65, 65], dt.bfloat16, tag="vt2", name="vt2")
        nc.vector.tensor_copy(out=vt[0][:], in_=att[:, 0:65])
        nc.vector.tensor_copy(out=vt[1][:], in_=att[:, 65:130])
        nc.vector.tensor_copy(out=vt2[:, :], in_=att[0:65, 130:195])
        # broadcast per-sample sum-of-exp to all 128 partitions (the output
        # chunks use partition p = h*64 + s to halve per-partition DMA bytes)
        sinv = csp.tile([128, 16], dt.float32, tag="sinv", name="sinv")
        if TP64:
            scol = csp.tile([SPC, 1], dt.float32, tag="scol", name="scol")
            nc.vector.tensor_copy(out=scol[:], in_=att[0:SPC, 195:196])
            pS = pfp.tile([128, 512], dt.float32, tag="psf", name="psf")
            nc.tensor.matmul(out=pS[:, 0:1], lhsT=iddup[:, :], rhs=scol[:],
                             start=True, stop=True)
            nc.vector.reciprocal(sinv[:, 0:1], pS[:, 0:1])
        else:
            nc.vector.reciprocal(sinv[0:SPC, 0:1], att[0:SPC, 195:196])

        # stream outw now that the span pools are freed; out chunk j covers
        # output cols [j*1000, (j+1)*1000) as [128 = (half, sample), 500]
        ow3 = dr["outw"].ap()[:, :].rearrange("p (c n) -> p c n", c=3)
        with tc.tile_pool(name="rtp", bufs=10) as rtp, \
             tc.tile_pool(name="otp", bufs=3) as otp:
            rts = []
            for j in range(OUT_D // OW):
                rt = rtp.tile([128, 3 * OW], dt.bfloat16, tag="rt", name="rt")
                r3 = rt[:].rearrange("p (c n) -> p c n", c=3)
                nc.sync.dma_start(r3[:, :, :], ow3[:, :, j * OW : (j + 1) * OW])
                rts.append(rt)
            for j in range(OUT_D // OW):
                r3 = rts[j][:].rearrange("p (c n) -> p c n", c=3)
                if TP64:
                    ot = otp.tile([128, 500], dt.float32, tag="ot", name="ot")
                    pot = pfp.tile([128, 512], dt.float32, tag="psf",
                                   name="psf")
                    for h in (0, 1):
                        po = pot[64 * h : 64 * h + SPC, 0:500]
                        hs = slice(h * 500, h * 500 + 500)
                        nc.tensor.matmul(out=po, lhsT=vt[0][:, 0:SPC],
                                         rhs=r3[:, 0, hs],
                                         start=True, stop=False)
                        nc.tensor.matmul(out=po, lhsT=vt[1][:, 0:SPC],
                                         rhs=r3[:, 1, hs],
                                         start=False, stop=False)
                        nc.tensor.matmul(out=po, lhsT=vt2[:, 0:SPC],
                                         rhs=r3[0:65, 2, hs],
                                         start=False, stop=True)
                    nc.vector.tensor_scalar(
                        out=ot[:], in0=pot[:, 0:500],
                        scalar1=sinv[:, 0:1], scalar2=None, op0=ALU.mult)
                    for h in (0, 1):
                        nc.sync.dma_start(
                            out_d.ap()[:, j * OW + h * 500 :
                                       j * OW + h * 500 + 500],
                            ot[64 * h : 64 * h + SPC, :])
                else:
                    ot = otp.tile([SPC, OW], dt.float32, tag="ot", name="ot")
                    for s5 in range(OW // 500):
                        c0 = s5 * 500
                        pot = pfp.tile([128, 512], dt.float32, tag="psf",
                                       name="psf")
                        po = pot[0:SPC, 0:500]
                        nc.tensor.matmul(out=po, lhsT=vt[0][:, 0:SPC],
                                         rhs=r3[:, 0, c0 : c0 + 500],
                                         start=True, stop=False)
                        nc.tensor.matmul(out=po, lhsT=vt[1][:, 0:SPC],
                                         rhs=r3[:, 1, c0 : c0 + 500],
                                         start=False, stop=False)
                        nc.tensor.matmul(out=po, lhsT=vt2[:, 0:SPC],
                                         rhs=r3[0:65, 2, c0 : c0 + 500],
                                         start=False, stop=True)
                        nc.vector.tensor_scalar(
                            out=ot[:, c0 : c0 + 500], in0=po,
                            scalar1=sinv[0:SPC, 0:1], scalar2=None,
                            op0=ALU.mult)
                    nc.sync.dma_start(out_d.ap()[:, j * OW : (j + 1) * OW],
                                      ot[:])

    nc.compile()
    return nc


# ---- top-level entry ------------------------------------------------------

_CACHE = {}


def _build_in_maps(meta, cores, sh):
    in_maps = []
    for d in cores:
        m = {
            "sub_lo": sh["sub_lo"], "sub_hi": sh["sub_hi"],
            "node_t": sh["node_t"],
            "ramp": sh["ramp"], "a_bc": sh["a_bc"],
            "fcw": sh["fcw"], "outw": sh["outw"], "iddup": sh["iddup"],
            "xidx": d["xidx"], "v0idx": d["v0idx"], "v8idx": d["v8idx"],
            "sampid": d["sampid"],
        }
        for dd in ("f", "b"):
            m[f"tbl_{dd}"] = sh[f"tbl_{dd}"]
            m[f"tbl_{dd}h"] = sh[f"tbl_{dd}h"]
            m[f"tbl_{dd}c"] = sh[f"tbl_{dd}c"]
            m[f"wih_{dd}"] = sh[f"wih_{dd}"]
            m[f"whh_{dd}"] = sh[f"whh_{dd}"]
            m[f"bias_{dd}"] = sh[f"bias_{dd}"]
        for side in ("ll", "rl"):
            for cls in ("lo", "hi"):
                m[f"{side}_{cls}_idx"] = d[f"{side}_{cls}_idx"]
                m[f"{side}_{cls}_ctx"] = d[f"{side}_{cls}_ctx"]
        in_maps.append(m)
    return in_maps


def _shapes_of(in_map):
    import concourse.mybir as mybir
    dt = mybir.dt
    np2my = {
        np.dtype(np.float32): dt.float32,
        np.dtype(BF16): dt.bfloat16,
        np.dtype(np.int16): dt.int16,
    }
    return [(k, list(v.shape), np2my[v.dtype]) for k, v in sorted(in_map.items())]


def kernel(**inputs):
    from concourse.bass_utils import run_bass_kernel_spmd

    inp = {k: np.asarray(v) for k, v in inputs.items()}
    meta, cores = prep_all(inp)
    sh = prep_shared(inp)
    in_maps = _build_in_maps(meta, cores, sh)
    shapes = _shapes_of(in_maps[0])
    key = (str(shapes) + str(meta["np"]) + f"{GX}{HCMERGE}{ATT_PSUM}{TP64}{XSTREAM}{TTR}{ILV}"
           + str({k: v.tolist() for k, v in meta["nb"].items()}))
    if key not in _CACHE:
        _CACHE[key] = build_nc(meta, shapes)
    nc = _CACHE[key]
    res = run_bass_kernel_spmd(nc, in_maps, core_ids=list(range(N_CORES)))
    return np.concatenate([res.results[c]["out"] for c in range(N_CORES)], axis=0)

